# revision 1
# baseline (speedup 1.0000x reference)
"""DEMONet 3-layer GNN message-passing kernel for 8x Trainium2 NeuronCores.

Math per layer (verified against reference):
    deg[i]   = #edges with src == i          (no deg-0 nodes in this data)
    nm       = segment_sum(h[dst], src) / deg
    out      = elu(h @ (Wg + Ws) + nm @ Wl + b)        (b == 0 here)

Sharding: nodes row-partitioned across 8 cores (12.5k real + pad -> 12800
per core).  Edges live with their src node's core.  Per layer, each core
computes h for its own nodes, then an AllGather of the bf16 h rows
rebuilds the full gather table on every core.

Neighbour rows are fetched with indirect_dma_start, one call per
128-edge chunk (HW semantics: one gathered row per partition).  A
host-side bin-packing assigns nodes to 128-node tiles such that every
tile needs exactly CPT 128-edge chunks (near-zero slot padding).

Segment-sum on device: gathered rows X_k [128e, 64] (bf16) are combined
on the TensorEngine with a 0/1 indicator S_k [128e, 128n]
(S[e,i] = 1 iff local_src[e] == i), accumulating PSUM[128n, 64] over the
tile's chunks.  S is built with a single DVE is_equal op per supertile.
"""

import os
import numpy as np
import ml_dtypes

import concourse.bass as bass
import concourse.bacc as bacc
import concourse.mybir as mybir
import concourse.tile as tile
from concourse.bass_utils import run_bass_kernel_spmd
from concourse.masks import make_identity

F32 = mybir.dt.float32
BF16 = mybir.dt.bfloat16
I32 = mybir.dt.int32
I16 = mybir.dt.int16
BF_NP = ml_dtypes.bfloat16

P = 128   # partitions / tile node count / chunk edge count
D = 64    # feature dim
DP = 128  # padded feature width of the gather table (256B rows)


class Cfg:
    def __init__(self, n_nodes, n_cores, npc_raw, npc, spt, nb, ctb,
                 n_layers=3):
        self.n_nodes = n_nodes
        self.n_cores = n_cores
        self.npc_raw = npc_raw
        self.npc = npc                  # padded nodes per core
        self.tpc = npc // P             # tiles per core
        self.spt = spt                  # tiles per supertile
        self.nst = self.tpc // spt
        self.nb = nb                    # index buckets (core-aligned)
        self.ctb = ctb                  # chunks per (tile, bucket)
        self.cpt = nb * ctb             # chunks per tile
        self.j = spt * self.cpt         # chunks per supertile
        self.ntot = n_cores * npc
        self.bs = self.ntot // nb       # bucket size (rows); must be < 32768
        assert self.bs <= 32768
        assert (n_cores * npc) % nb == 0 and npc * (n_cores // nb) == self.bs
        self.n_layers = n_layers


def _pack_core(sizes, tpc, cap):
    """Assign nodes (rows of `sizes` [n,nb]) to tpc tiles of 128 slots s.t.
    per-tile per-bucket sums <= cap.  Returns tile index per node."""
    n, nbk = sizes.shape
    order = np.argsort(-sizes.sum(1), kind="stable")
    rem = np.full((tpc, nbk), cap, np.int64)
    slots = np.full(tpc, P)
    assign = np.full(n, -1, np.int32)
    for i in order:
        s = sizes[i]
        cand = (slots > 0) & np.all(rem >= s, axis=1)
        if not cand.any():
            raise RuntimeError("node packing failed; increase ctb")
        scores = (rem - s).min(1).astype(np.float64) + 0.001 * slots
        scores[~cand] = -1e18
        t = int(np.argmax(scores))
        rem[t] -= s
        slots[t] -= 1
        assign[i] = t
    return assign


def prep_host(x, edge_index, cfg: Cfg):
    N = cfg.n_nodes
    NC, NPC_RAW, NPC, TPC, SPT, NB, CTB = (
        cfg.n_cores, cfg.npc_raw, cfg.npc, cfg.tpc, cfg.spt, cfg.nb, cfg.ctb)
    NST, CPT, J = cfg.nst, cfg.cpt, cfg.j
    src = np.asarray(edge_index[0], dtype=np.int64)
    dst = np.asarray(edge_index[1], dtype=np.int64)
    E = src.shape[0]

    deg = np.bincount(src, minlength=N)
    if deg.min() == 0:
        raise NotImplementedError(
            "deg-0 nodes present; the simplified Wg+Ws fusion is invalid")
    inv_deg = (1.0 / deg).astype(np.float32)

    c_src = np.minimum(src // NPC_RAW, NC - 1)
    c_dst = np.minimum(dst // NPC_RAW, NC - 1)
    bucket = c_dst // (NC // NB)

    # per-node out-degree per bucket, then pack nodes into tiles
    nbcnt = np.zeros((N, NB), np.int32)
    np.add.at(nbcnt, (src, bucket), 1)
    perm_pos = np.zeros(N, np.int64)      # orig id -> position within core
    for c in range(NC):
        lo, hi = c * NPC_RAW, min((c + 1) * NPC_RAW, N)
        n_local = hi - lo
        assign = _pack_core(nbcnt[lo:hi], TPC, CTB * P)
        # position within tile: stable order of placement
        order_t = np.argsort(assign, kind="stable")
        within = np.arange(n_local) - np.searchsorted(
            assign[order_t], assign[order_t])
        pos = np.empty(n_local, np.int64)
        pos[order_t] = assign[order_t] * P + within
        perm_pos[lo:hi] = pos
    gpos = np.minimum(np.arange(N) // NPC_RAW, NC - 1) * NPC + perm_pos

    pdst = gpos[dst]                      # permuted global dst id
    lsrc_tile = (perm_pos[src] % P).astype(np.int16)
    tile_of_src = perm_pos[src] // P      # tile within core
    st_of_src = tile_of_src // SPT
    t_in_st = tile_of_src % SPT

    # slot assignment: group by (core, st, bucket, tile-in-st)
    key = ((c_src * NST + st_of_src) * NB + bucket) * SPT + t_in_st
    n_groups = NC * NST * NB * SPT
    counts = np.bincount(key, minlength=n_groups)
    assert counts.max() <= CTB * P, (counts.max(), CTB * P)
    order = np.argsort(key, kind="stable")
    starts = np.zeros(n_groups + 1, np.int64)
    np.cumsum(counts, out=starts[1:])
    q = np.arange(E) - starts[key[order]]     # position within group
    ks = key[order]
    g_c = ks // (NST * NB * SPT)
    g_st = (ks // (NB * SPT)) % NST
    g_b = (ks // SPT) % NB
    g_t = ks % SPT
    chunk = g_b * (SPT * CTB) + g_t * CTB + q // P   # chunk within supertile
    p = q % P
    slot_i = (g_t * CTB + q // P) * P + p            # index within bucket region

    lsrc_arr = np.full((NC, NST, P, J), 300, np.int16)
    lsrc_arr[g_c, g_st, p, chunk] = lsrc_tile[order]

    gidx = np.zeros((NC, NST, P, J), np.int32)
    gidx[g_c, g_st, p, chunk] = pdst[order].astype(np.int32)

    # padded per-core node arrays (in permuted order)
    x = np.asarray(x, dtype=np.float32)
    x_pad = np.zeros((cfg.ntot, D), np.float32)
    invdeg_pad = np.zeros(cfg.ntot, np.float32)
    x_pad[gpos] = x
    invdeg_pad[gpos] = inv_deg
    xfull_bf = np.ascontiguousarray(x_pad.astype(BF_NP))

    per_core = []
    for c in range(NC):
        xs = x_pad[c * NPC:(c + 1) * NPC]
        per_core.append(dict(
            x_ownT=np.ascontiguousarray(xs.T),                    # [64, NPC]
            xfull_bf=xfull_bf,                                    # [NTOT, DP]
            gidx=np.ascontiguousarray(gidx[c]),                   # [NST,128,J]
            lsrc=np.ascontiguousarray(lsrc_arr[c]),               # [NST,128,J]
            invdegT=np.ascontiguousarray(
                invdeg_pad[c * NPC:(c + 1) * NPC].reshape(TPC, P).T),
        ))
    return per_core, perm_pos


def build_program(nc, cfg: Cfg, tc=None):
    NPC, NTOT, NST, SPT, NB, CTB, J, TPC = (
        cfg.npc, cfg.ntot, cfg.nst, cfg.spt, cfg.nb, cfg.ctb, cfg.j, cfg.tpc)
    NL = cfg.n_layers
    IDXW = NB * SPT * CTB * P // 16

    x_ownT = nc.dram_tensor("x_ownT", [D, NPC], F32, kind="ExternalInput")
    xfull_bf = nc.dram_tensor("xfull_bf", [NTOT, D], BF16, kind="ExternalInput")
    gidx = nc.dram_tensor("gidx", [NST, P, J], I32, kind="ExternalInput")
    lsrc = nc.dram_tensor("lsrc", [NST, P, J], I16, kind="ExternalInput")
    invdegT = nc.dram_tensor("invdegT", [P, TPC], F32, kind="ExternalInput")
    w_gs = [nc.dram_tensor(f"w_gs{L}", [D, D], F32, kind="ExternalInput")
            for L in range(NL)]
    w_l = [nc.dram_tensor(f"w_l{L}", [D, D], F32, kind="ExternalInput")
           for L in range(NL)]
    out_own = nc.dram_tensor("out_own", [NPC, D], F32, kind="ExternalOutput")

    hT_own = [nc.dram_tensor(f"hT_own{L}", [D, NPC], F32, kind="Internal")
              for L in range(NL - 1)]
    ag_in = [nc.dram_tensor(f"ag_in{L}", [NPC, D], BF16, kind="Internal")
             for L in range(NL - 1)]
    h_full = [nc.dram_tensor(f"h_full{L}", [NTOT, D], BF16, kind="Internal",
                             addr_space="Shared" if cfg.n_cores > 4 else "Local")
              for L in range(NL - 1)]

    own_ctx = tc is None
    if own_ctx:
        tc = tile.TileContext(nc)
        tc.__enter__()
    try:
        _emit(nc, tc, cfg, locals())
    finally:
        if own_ctx:
            tc.__exit__(None, None, None)
    return nc


def _emit(nc, tc, cfg: Cfg, T):
    NPC, NTOT, NST, SPT, NB, CTB, J, TPC, NL = (
        cfg.npc, cfg.ntot, cfg.nst, cfg.spt, cfg.nb, cfg.ctb, cfg.j, cfg.tpc,
        cfg.n_layers)
    x_ownT, xfull_bf, gidx, lsrc, invdegT = (
        T["x_ownT"], T["xfull_bf"], T["gidx"], T["lsrc"], T["invdegT"])
    w_gs, w_l, out_own = T["w_gs"], T["w_l"], T["out_own"]
    hT_own, ag_in, h_full = T["hT_own"], T["ag_in"], T["h_full"]
    RCH = SPT * CTB               # chunks per bucket region

    with (
        tc.tile_pool(name="const", bufs=1) as constp,
        tc.tile_pool(name="io", bufs=3) as iop,
        tc.tile_pool(name="big", bufs=2) as bigp,
        tc.tile_pool(name="small", bufs=4) as smallp,
        tc.tile_pool(name="psA", bufs=2, space="PSUM") as psA,
        tc.tile_pool(name="psB", bufs=2, space="PSUM") as psB,
        tc.tile_pool(name="psC", bufs=2, space="PSUM") as psC,
        tc.tile_pool(name="psD", bufs=2, space="PSUM") as psD,
    ):
        ident = constp.tile([P, P], F32, name="ident")
        make_identity(nc, ident[:])
        iota16 = constp.tile([P, P], I16, name="iota16")
        nc.gpsimd.iota(iota16[:], pattern=[[1, P]], base=0, channel_multiplier=0)
        invdeg_sb = constp.tile([P, TPC], F32, name="invdeg_sb")
        nc.sync.dma_start(invdeg_sb[:], invdegT[:])
        wgs_sb, wl_sb = [], []
        for L in range(NL):
            wg_t = constp.tile([D, D], F32, name=f"wgs_sb{L}")
            nc.sync.dma_start(wg_t[:], w_gs[L][:])
            wgs_sb.append(wg_t)
            wl_t = constp.tile([D, D], F32, name=f"wl_sb{L}")
            nc.sync.dma_start(wl_t[:], w_l[L][:])
            wl_sb.append(wl_t)

        for L in range(NL):
            table = xfull_bf if L == 0 else h_full[L - 1]
            srcT = x_ownT if L == 0 else hT_own[L - 1]
            last = L == NL - 1
            for s in range(NST):
                idx_t = iop.tile([P, J], I32, tag="idx", name=f"idx_{L}_{s}")
                nc.sync.dma_start(idx_t[:], gidx[s, :, :])
                lsrc_t = iop.tile([P, J], I16, tag="lsrc", name=f"lsr_{L}_{s}")
                nc.sync.dma_start(lsrc_t[:], lsrc[s, :, :])

                X = bigp.tile([P, J * D], BF16, tag="X", name=f"X_{L}_{s}")
                X3 = X[:].rearrange("p (c e) -> p c e", e=D)
                for c in range(J):
                    nc.gpsimd.indirect_dma_start(
                        out=X3[:, c, :], out_offset=None,
                        in_=table[:],
                        in_offset=bass.IndirectOffsetOnAxis(
                            ap=idx_t[:, c:c + 1], axis=0),
                    )

                S = bigp.tile([P, J * P], BF16, tag="S", name=f"S_{L}_{s}")
                nc.vector.tensor_tensor(
                    out=S[:].rearrange("p (j i) -> p j i", i=P),
                    in0=lsrc_t[:, :, None].to_broadcast([P, J, P]),
                    in1=iota16[:, None, :].to_broadcast([P, J, P]),
                    op=mybir.AluOpType.is_equal,
                )

                hT_st = bigp.tile([D, SPT * P], F32, tag="hT", name=f"hT_{L}_{s}")
                nc.sync.dma_start(hT_st[:], srcT[:, s * SPT * P:(s + 1) * SPT * P])

                hnew = bigp.tile([P, SPT * D], F32, tag="hnew", name=f"hn_{L}_{s}")
                if not last:
                    hbf = bigp.tile([P, SPT * D], BF16, tag="hbf",
                                    name=f"hb_{L}_{s}")
                    hnT = bigp.tile([D, SPT * P], F32, tag="hnT", name=f"hT2_{L}_{s}")

                for t in range(SPT):
                    g_t = s * SPT + t
                    chunks = [b * RCH + t * CTB + k
                              for b in range(NB) for k in range(CTB)]
                    nmP = psA.tile([P, D], F32, tag="nmP", name=f"nmP_{L}_{s}_{t}")
                    for ci, ch in enumerate(chunks):
                        nc.tensor.matmul(
                            nmP[:],
                            lhsT=S[:, ch * P:(ch + 1) * P],
                            rhs=X3[:, ch, :],
                            start=(ci == 0), stop=(ci == len(chunks) - 1),
                        )
                    nm = smallp.tile([P, D], F32, tag="nm", name=f"nm_{L}_{s}_{t}")
                    nc.vector.tensor_scalar_mul(
                        nm[:], nmP[:], invdeg_sb[:, g_t:g_t + 1])
                    nmTP = psB.tile([D, P], F32, tag="nmTP", name=f"nmTP_{L}_{s}_{t}")
                    nc.tensor.transpose(nmTP[:], nm[:], ident[:])
                    nmT = smallp.tile([D, P], F32, tag="nmT", name=f"nmT_{L}_{s}_{t}")
                    nc.vector.tensor_copy(nmT[:], nmTP[:])
                    outP = psC.tile([P, D], F32, tag="outP", name=f"oP_{L}_{s}_{t}")
                    nc.tensor.matmul(
                        outP[:], lhsT=hT_st[:, t * P:(t + 1) * P],
                        rhs=wgs_sb[L][:], start=True, stop=False)
                    nc.tensor.matmul(
                        outP[:], lhsT=nmT[:], rhs=wl_sb[L][:],
                        start=False, stop=True)
                    # ELU: out = (max(x,0)-1) + exp(min(x,0))
                    lo = smallp.tile([P, D], F32, tag="lo", name=f"lo_{L}_{s}_{t}")
                    nc.vector.tensor_scalar_min(lo[:], outP[:], 0.0)
                    ex = smallp.tile([P, D], F32, tag="ex", name=f"ex_{L}_{s}_{t}")
                    nc.scalar.activation(ex[:], lo[:],
                                         mybir.ActivationFunctionType.Exp)
                    hi1 = smallp.tile([P, D], F32, tag="hi1", name=f"hi_{L}_{s}_{t}")
                    nc.vector.tensor_scalar(
                        hi1[:], outP[:], 0.0, 1.0,
                        op0=mybir.AluOpType.max, op1=mybir.AluOpType.subtract)
                    nc.vector.tensor_add(
                        hnew[:, t * D:(t + 1) * D], ex[:], hi1[:])
                    if not last:
                        nc.vector.tensor_copy(
                            hbf[:, t * D:(t + 1) * D],
                            hnew[:, t * D:(t + 1) * D])
                        hnTP = psD.tile([D, P], F32, tag="hnTP",
                                        name=f"hnTP_{L}_{s}_{t}")
                        nc.tensor.transpose(
                            hnTP[:], hnew[:, t * D:(t + 1) * D], ident[:])
                        nc.vector.tensor_copy(
                            hnT[:, t * P:(t + 1) * P], hnTP[:])

                if last:
                    dst_rows = out_own.rearrange(
                        "(s t p) d -> s p t d", s=NST, t=SPT, p=P)
                    nc.sync.dma_start(
                        dst_rows[s],
                        hnew[:].rearrange("p (t d) -> p t d", d=D))
                else:
                    bf_rows = ag_in[L].rearrange(
                        "(s t p) d -> s p t d", s=NST, t=SPT, p=P)
                    nc.sync.dma_start(
                        bf_rows[s],
                        hbf[:].rearrange("p (t d) -> p t d", d=D))
                    nc.sync.dma_start(
                        hT_own[L][:, s * SPT * P:(s + 1) * SPT * P], hnT[:])
            if not last:
                nc.gpsimd.collective_compute(
                    "AllGather",
                    mybir.AluOpType.bypass,
                    replica_groups=[list(range(cfg.n_cores))],
                    ins=[ag_in[L][:]],
                    outs=[h_full[L][:]],
                )


def _make_cfg_full():
    return Cfg(n_nodes=100000, n_cores=8, npc_raw=12500, npc=12800,
               spt=4, nb=4, ctb=3)


def kernel(**inputs):
    cfg = _make_cfg_full()
    x = np.asarray(inputs["x"], np.float32)
    ei = np.asarray(inputs["edge_index"])
    Ws = []
    for L, (a, b, c, bias) in enumerate(
            [("Wg1", "Wl1", "Ws1", "b1"), ("Wg2", "Wl2", "Ws2", "b2"),
             ("Wgo", "Wlo", "Wso", "bo")]):
        bv = np.asarray(inputs[bias], np.float32)
        assert np.all(bv == 0.0), "nonzero bias not supported by this build"
        Ws.append((np.asarray(inputs[a], np.float32) +
                   np.asarray(inputs[c], np.float32),
                   np.asarray(inputs[b], np.float32)))

    per_core, perm_pos = prep_host(x, ei, cfg)

    nc = bacc.Bacc("TRN2", target_bir_lowering=False, debug=False,
                   enable_asserts=False, num_devices=cfg.n_cores)
    build_program(nc, cfg)
    nc.compile()

    in_maps = []
    for c in range(cfg.n_cores):
        m = dict(per_core[c])
        for L in range(3):
            m[f"w_gs{L}"] = Ws[L][0]
            m[f"w_l{L}"] = Ws[L][1]
        in_maps.append(m)

    res = run_bass_kernel_spmd(
        nc, in_maps, core_ids=list(range(cfg.n_cores)),
        trace=bool(int(os.environ.get("GNN_TRACE", "0"))),
    )
    full = np.zeros((cfg.n_nodes, D), np.float32)
    for c in range(cfg.n_cores):
        lo = c * cfg.npc_raw
        hi = min((c + 1) * cfg.npc_raw, cfg.n_nodes)
        full[lo:hi] = res.results[c]["out_own"][perm_pos[lo:hi]]
    kernel.last_results = res
    return full.astype(np.float32)



# revision 6
# speedup vs baseline: 1.3092x; 1.3092x over previous
"""DEMONet 3-layer GNN message-passing kernel for 8x Trainium2 NeuronCores.

Math per layer (verified against reference; all nodes have deg > 0):
    nm   = segment_sum(h[dst], src) / deg
    out  = elu(h @ (Wg + Ws) + nm @ Wl + b)          (b == 0 here)

Key structure (v2):
  * Nodes row-partitioned across 8 cores (12.5k real + pad -> 12800 per
    core).  Edges live with their src node's core.
  * The gather table for layer L holds rows (h_{L-1} @ Wl_L) in bf16,
    padded to 128 cols (256B rows).  Folding Wl into the table lets the
    on-device segment-sum produce nm @ Wl directly, removing the per-tile
    transpose + second matmul.  Layer 0's table (x @ Wl1) is computed on
    the host; tables for layers 1,2 are produced on device and AllGathered.
  * Neighbour rows are fetched with gpsimd.dma_gather: ONE instruction per
    (supertile, bucket) gathers 1536 rows (vs. indirect_dma_start's 128),
    cutting SWDGE fixed overhead ~12x.  Indices are int16 relative to one
    of NB=4 node buckets (25600 rows < 32768).
  * Segment-sum on the TensorEngine: gathered rows X_ch [128e, 64] (bf16)
    are combined with a 0/1 indicator S_ch [128e, 128n] accumulating
    PSUM[128n, 64] over a tile's 12 chunks.  S is built once per
    (layer, supertile) with a single DVE is_equal.
  * h kept transposed [64, NPC] in bf16 in SBUF across layers (no DRAM
    roundtrip); per tile: g-term matmul, fused (SU*invdeg)+g, ELU chain,
    transpose to update hT, and the next-layer table row matmul.
"""

import os
import numpy as np
import ml_dtypes

import concourse.bass as bass
import concourse.bacc as bacc
import concourse.mybir as mybir
import concourse.tile as tile
from concourse.bass_utils import run_bass_kernel_spmd
from concourse.masks import make_identity

F32 = mybir.dt.float32
BF16 = mybir.dt.bfloat16
I32 = mybir.dt.int32
I16 = mybir.dt.int16
BF_NP = ml_dtypes.bfloat16

P = 128   # partitions / tile node count / chunk edge count
D = 64    # feature dim
DP = 128  # padded feature width of the gather table (256B rows)
GMAX = 8  # max chunks (of 128 rows) per dma_gather instruction (1024 rows)


class Cfg:
    def __init__(self, n_nodes, n_cores, npc_raw, npc, spt, nb, ctb,
                 n_layers=3):
        self.n_nodes = n_nodes
        self.n_cores = n_cores
        self.npc_raw = npc_raw
        self.npc = npc                  # padded nodes per core
        self.tpc = npc // P             # tiles per core
        self.spt = spt                  # tiles per supertile
        self.nst = self.tpc // spt
        self.nb = nb                    # index buckets (core-aligned)
        self.ctb = ctb                  # chunks per (tile, bucket)
        self.cpt = nb * ctb             # chunks per tile
        self.j = spt * self.cpt         # chunks per supertile
        self.ntot = n_cores * npc
        self.bs = self.ntot // nb       # bucket size (rows); must be < 32768
        assert self.bs <= 32768
        assert (n_cores * npc) % nb == 0 and npc * (n_cores // nb) == self.bs
        self.n_layers = n_layers


def _pack_core(sizes, tpc, cap):
    """Assign nodes (rows of `sizes` [n,nb]) to tpc tiles of 128 slots s.t.
    per-tile per-bucket sums <= cap.  Returns tile index per node."""
    n, nbk = sizes.shape
    order = np.argsort(-sizes.sum(1), kind="stable")
    rem = np.full((tpc, nbk), cap, np.int64)
    slots = np.full(tpc, P)
    assign = np.full(n, -1, np.int32)
    for i in order:
        s = sizes[i]
        cand = (slots > 0) & np.all(rem >= s, axis=1)
        if not cand.any():
            raise RuntimeError("node packing failed; increase ctb")
        scores = (rem - s).min(1).astype(np.float64) + 0.001 * slots
        scores[~cand] = -1e18
        t = int(np.argmax(scores))
        rem[t] -= s
        slots[t] -= 1
        assign[i] = t
    return assign


def prep_host(x, edge_index, Wl1, cfg: Cfg):
    N = cfg.n_nodes
    NC, NPC_RAW, NPC, TPC, SPT, NB, CTB = (
        cfg.n_cores, cfg.npc_raw, cfg.npc, cfg.tpc, cfg.spt, cfg.nb, cfg.ctb)
    NST, CPT, J = cfg.nst, cfg.cpt, cfg.j
    RCH = SPT * CTB
    BS = cfg.bs
    src = np.asarray(edge_index[0], dtype=np.int64)
    dst = np.asarray(edge_index[1], dtype=np.int64)
    E = src.shape[0]

    deg = np.bincount(src, minlength=N)
    if deg.min() == 0:
        raise NotImplementedError(
            "deg-0 nodes present; the simplified Wg+Ws fusion is invalid")
    inv_deg = (1.0 / deg).astype(np.float32)

    c_src = np.minimum(src // NPC_RAW, NC - 1)
    c_dst = np.minimum(dst // NPC_RAW, NC - 1)
    bucket = c_dst // (NC // NB)

    # per-node out-degree per bucket, then pack nodes into tiles
    nbcnt = np.zeros((N, NB), np.int32)
    np.add.at(nbcnt, (src, bucket), 1)
    perm_pos = np.zeros(N, np.int64)      # orig id -> position within core
    for c in range(NC):
        lo, hi = c * NPC_RAW, min((c + 1) * NPC_RAW, N)
        n_local = hi - lo
        assign = _pack_core(nbcnt[lo:hi], TPC, CTB * P)
        order_t = np.argsort(assign, kind="stable")
        within = np.arange(n_local) - np.searchsorted(
            assign[order_t], assign[order_t])
        pos = np.empty(n_local, np.int64)
        pos[order_t] = assign[order_t] * P + within
        perm_pos[lo:hi] = pos
    gpos = np.minimum(np.arange(N) // NPC_RAW, NC - 1) * NPC + perm_pos

    pdst = gpos[dst]                      # permuted global dst id
    lsrc_tile = (perm_pos[src] % P).astype(np.int16)
    tile_of_src = perm_pos[src] // P      # tile within core
    st_of_src = tile_of_src // SPT
    t_in_st = tile_of_src % SPT

    # slot assignment: group by (core, st, bucket, tile-in-st)
    key = ((c_src * NST + st_of_src) * NB + bucket) * SPT + t_in_st
    n_groups = NC * NST * NB * SPT
    counts = np.bincount(key, minlength=n_groups)
    assert counts.max() <= CTB * P, (counts.max(), CTB * P)
    order = np.argsort(key, kind="stable")
    starts = np.zeros(n_groups + 1, np.int64)
    np.cumsum(counts, out=starts[1:])
    q = np.arange(E) - starts[key[order]]     # position within group
    ks = key[order]
    g_c = ks // (NST * NB * SPT)
    g_st = (ks // (NB * SPT)) % NST
    g_b = (ks // SPT) % NB
    g_t = ks % SPT
    chunk = g_b * RCH + g_t * CTB + q // P   # chunk within supertile
    p = q % P

    lsrc_arr = np.full((NC, NST, P, J), 300, np.int16)
    lsrc_arr[g_c, g_st, p, chunk] = lsrc_tile[order]

    # bucket-relative int16 indices; pads point at bucket row 0 (their
    # contribution is killed by S == 0)
    rel = np.zeros((NC, NST, P, J), np.int16)
    rel[g_c, g_st, p, chunk] = (pdst[order] - g_b * BS).astype(np.int16)

    # dma_gather wrapped layout: idx i (-> partition i%128, chunk i//128 of
    # the output) is read from idxs[i%16, i//16]; replicate over 8 groups.
    W16 = RCH * P // 16                   # 96 idx columns per bucket
    blocks = rel.reshape(NC, NST, P, NB, RCH)
    flat = blocks.transpose(0, 1, 3, 4, 2).reshape(NC, NST, NB, RCH * P)
    w = flat.reshape(NC, NST, NB, W16, 16).transpose(0, 1, 2, 4, 3)
    wfull = np.broadcast_to(w[:, :, :, None, :, :],
                            (NC, NST, NB, 8, 16, W16))
    gidx16 = np.ascontiguousarray(
        wfull.reshape(NC, NST, NB, P, W16).transpose(0, 1, 3, 2, 4)
        .reshape(NC, NST, P, NB * W16))

    # padded per-core node arrays (in permuted order)
    x = np.asarray(x, dtype=np.float32)
    x_pad = np.zeros((cfg.ntot, D), np.float32)
    invdeg_pad = np.zeros(cfg.ntot, np.float32)
    x_pad[gpos] = x
    invdeg_pad[gpos] = inv_deg

    # layer-0 gather table: (x @ Wl1) in bf16, 256B rows
    t0 = (x_pad @ np.asarray(Wl1, np.float32)).astype(BF_NP)
    table0 = np.zeros((cfg.ntot, DP), BF_NP)
    table0[:, :D] = t0

    per_core = []
    for c in range(NC):
        xs = x_pad[c * NPC:(c + 1) * NPC]
        per_core.append(dict(
            x_ownT=np.ascontiguousarray(xs.T.astype(BF_NP)),      # [64, NPC]
            table0=table0,                                        # [NTOT, DP]
            gidx16=np.ascontiguousarray(gidx16[c]),               # [NST,128,NB*96]
            lsrc=np.ascontiguousarray(lsrc_arr[c]),               # [NST,128,J]
            invdegT=np.ascontiguousarray(
                invdeg_pad[c * NPC:(c + 1) * NPC].reshape(TPC, P).T),
        ))
    return per_core, perm_pos


def build_program(nc, cfg: Cfg, tc=None):
    NPC, NTOT, NST, SPT, NB, CTB, J, TPC = (
        cfg.npc, cfg.ntot, cfg.nst, cfg.spt, cfg.nb, cfg.ctb, cfg.j, cfg.tpc)
    NL = cfg.n_layers
    RCH = SPT * CTB
    W16 = RCH * P // 16

    x_ownT = nc.dram_tensor("x_ownT", [D, NPC], BF16, kind="ExternalInput")
    table0 = nc.dram_tensor("table0", [NTOT, DP], BF16, kind="ExternalInput")
    gidx16 = nc.dram_tensor("gidx16", [NST, P, NB * W16], I16,
                            kind="ExternalInput")
    lsrc = nc.dram_tensor("lsrc", [NST, P, J], I16, kind="ExternalInput")
    invdegT = nc.dram_tensor("invdegT", [P, TPC], F32, kind="ExternalInput")
    w_gs = [nc.dram_tensor(f"w_gs{L}", [D, D], BF16, kind="ExternalInput")
            for L in range(NL)]
    w_ln = [nc.dram_tensor(f"w_ln{L}", [D, D], BF16, kind="ExternalInput")
            for L in range(NL - 1)]   # Wl of layer L+1
    out_own = nc.dram_tensor("out_own", [NPC, D], F32, kind="ExternalOutput")

    t2_own = [nc.dram_tensor(f"t2_own{L}", [NPC, DP], BF16, kind="Internal")
              for L in range(NL - 1)]
    h_full = [nc.dram_tensor(f"h_full{L}", [NTOT, DP], BF16, kind="Internal",
                             addr_space="Shared" if cfg.n_cores > 4 else "Local")
              for L in range(NL - 1)]

    own_ctx = tc is None
    if own_ctx:
        tc = tile.TileContext(nc)
        tc.__enter__()
    try:
        _emit(nc, tc, cfg, locals())
    finally:
        if own_ctx:
            tc.__exit__(None, None, None)
    return nc


def _emit(nc, tc, cfg: Cfg, T):
    NPC, NTOT, NST, SPT, NB, CTB, J, TPC, NL = (
        cfg.npc, cfg.ntot, cfg.nst, cfg.spt, cfg.nb, cfg.ctb, cfg.j, cfg.tpc,
        cfg.n_layers)
    x_ownT, table0, gidx16, lsrc, invdegT = (
        T["x_ownT"], T["table0"], T["gidx16"], T["lsrc"], T["invdegT"])
    w_gs, w_ln, out_own = T["w_gs"], T["w_ln"], T["out_own"]
    t2_own, h_full = T["t2_own"], T["h_full"]
    RCH = SPT * CTB               # chunks per bucket region
    BS = cfg.bs
    W16 = RCH * P // 16

    with (
        tc.tile_pool(name="const", bufs=1) as constp,
        tc.tile_pool(name="io", bufs=3) as iop,
        tc.tile_pool(name="big", bufs=2) as bigp,
        tc.tile_pool(name="small", bufs=4) as smallp,
        tc.tile_pool(name="psA", bufs=2, space="PSUM") as psA,
        tc.tile_pool(name="psB", bufs=2, space="PSUM") as psB,
        tc.tile_pool(name="psC", bufs=2, space="PSUM") as psC,
        tc.tile_pool(name="psD", bufs=2, space="PSUM") as psD,
    ):
        ident = constp.tile([P, P], BF16, name="ident")
        make_identity(nc, ident[:])
        iota16 = constp.tile([P, P], I16, name="iota16")
        nc.gpsimd.iota(iota16[:], pattern=[[1, P]], base=0, channel_multiplier=0)
        invdeg_sb = constp.tile([P, TPC], F32, name="invdeg_sb")
        nc.sync.dma_start(invdeg_sb[:], invdegT[:])
        wgs_sb, wln_sb = [], []
        for L in range(NL):
            wg_t = constp.tile([D, D], BF16, name=f"wgs_sb{L}")
            nc.sync.dma_start(wg_t[:], w_gs[L][:])
            wgs_sb.append(wg_t)
        for L in range(NL - 1):
            wl_t = constp.tile([D, D], BF16, name=f"wln_sb{L}")
            nc.sync.dma_start(wl_t[:], w_ln[L][:])
            wln_sb.append(wl_t)
        hT = [constp.tile([D, NPC], BF16, name=f"hT{i}") for i in range(2)]
        nc.sync.dma_start(hT[0][:], x_ownT[:])

        for L in range(NL):
            table = table0 if L == 0 else h_full[L - 1]
            hT_in, hT_out = hT[L % 2], hT[(L + 1) % 2]
            last = L == NL - 1
            for s in range(NST):
                idx_t = iop.tile([P, NB * W16], I16, tag="idx",
                                 name=f"idx_{L}_{s}")
                nc.sync.dma_start(idx_t[:], gidx16[s, :, :])
                lsrc_t = iop.tile([P, J], I16, tag="lsrc", name=f"lsr_{L}_{s}")
                nc.sync.dma_start(lsrc_t[:], lsrc[s, :, :])

                X = bigp.tile([P, J * DP], BF16, tag="X", name=f"X_{L}_{s}")
                X3 = X[:].rearrange("p (c e) -> p c e", e=DP)
                # HW limit: <= 1024 gathered rows (8 chunks) per dma_gather
                for b in range(NB):
                    for c0 in range(0, RCH, GMAX):
                        c1 = min(c0 + GMAX, RCH)
                        nidx = (c1 - c0) * P
                        nc.gpsimd.dma_gather(
                            out_ap=X3[:, b * RCH + c0:b * RCH + c1, :],
                            in_ap=table[b * BS:(b + 1) * BS, :],
                            idxs_ap=idx_t[:, b * W16 + c0 * (P // 16):
                                          b * W16 + c1 * (P // 16)],
                            num_idxs=nidx,
                            num_idxs_reg=nidx,
                            elem_size=DP,
                        )

                S = bigp.tile([P, J * P], BF16, tag="S", name=f"S_{L}_{s}")
                nc.vector.tensor_tensor(
                    out=S[:].rearrange("p (j i) -> p j i", i=P),
                    in0=lsrc_t[:, :, None].to_broadcast([P, J, P]),
                    in1=iota16[:, None, :].to_broadcast([P, J, P]),
                    op=mybir.AluOpType.is_equal,
                )

                if last:
                    hnew = bigp.tile([P, SPT * D], F32, tag="hnew",
                                     name=f"hn_{L}_{s}")
                else:
                    hnb = bigp.tile([P, SPT * D], BF16, tag="hnb",
                                    name=f"hb_{L}_{s}")
                    t2b = bigp.tile([P, SPT * DP], BF16, tag="t2b",
                                    name=f"t2_{L}_{s}")

                for t in range(SPT):
                    g_t = s * SPT + t
                    chunks = [b * RCH + t * CTB + k
                              for b in range(NB) for k in range(CTB)]
                    SU = psA.tile([P, D], F32, tag="SU", name=f"SU_{L}_{s}_{t}")
                    for ci, ch in enumerate(chunks):
                        nc.tensor.matmul(
                            SU[:],
                            lhsT=S[:, ch * P:(ch + 1) * P],
                            rhs=X3[:, ch, 0:D],
                            start=(ci == 0), stop=(ci == len(chunks) - 1),
                        )
                    gP = psC.tile([P, D], F32, tag="gP", name=f"gP_{L}_{s}_{t}")
                    nc.tensor.matmul(
                        gP[:], lhsT=hT_in[:, g_t * P:(g_t + 1) * P],
                        rhs=wgs_sb[L][:], start=True, stop=True)
                    # pre = SU * invdeg + g  (two ops: only one PSUM input
                    # allowed per DVE instruction)
                    e_sb = smallp.tile([P, D], F32, tag="e",
                                       name=f"e_{L}_{s}_{t}")
                    nc.vector.tensor_scalar_mul(
                        e_sb[:], SU[:], invdeg_sb[:, g_t:g_t + 1])
                    pre = smallp.tile([P, D], F32, tag="pre",
                                      name=f"pr_{L}_{s}_{t}")
                    nc.vector.tensor_add(pre[:], e_sb[:], gP[:])
                    # ELU: out = (max(x,0)-1) + exp(min(x,0))
                    lo = smallp.tile([P, D], F32, tag="lo", name=f"lo_{L}_{s}_{t}")
                    nc.vector.tensor_scalar_min(lo[:], pre[:], 0.0)
                    ex = smallp.tile([P, D], F32, tag="ex", name=f"ex_{L}_{s}_{t}")
                    nc.scalar.activation(ex[:], lo[:],
                                         mybir.ActivationFunctionType.Exp)
                    hi1 = smallp.tile([P, D], F32, tag="hi1",
                                      name=f"hi_{L}_{s}_{t}")
                    nc.vector.tensor_scalar(
                        hi1[:], pre[:], 0.0, 1.0,
                        op0=mybir.AluOpType.max, op1=mybir.AluOpType.subtract)
                    if last:
                        nc.vector.tensor_add(
                            hnew[:, t * D:(t + 1) * D], ex[:], hi1[:])
                    else:
                        nc.vector.tensor_add(
                            hnb[:, t * D:(t + 1) * D], ex[:], hi1[:])
                        hTP = psD.tile([D, P], BF16, tag="hTP",
                                       name=f"hTP_{L}_{s}_{t}")
                        nc.tensor.transpose(
                            hTP[:], hnb[:, t * D:(t + 1) * D], ident[:])
                        nc.vector.tensor_copy(
                            hT_out[:, g_t * P:(g_t + 1) * P], hTP[:])
                        t2P = psB.tile([P, D], F32, tag="t2P",
                                       name=f"t2P_{L}_{s}_{t}")
                        nc.tensor.matmul(
                            t2P[:], lhsT=hT_out[:, g_t * P:(g_t + 1) * P],
                            rhs=wln_sb[L][:], start=True, stop=True)
                        nc.vector.tensor_copy(
                            t2b[:, t * DP:t * DP + D], t2P[:])

                if last:
                    dst_rows = out_own.rearrange(
                        "(s t p) d -> s p t d", s=NST, t=SPT, p=P)
                    nc.sync.dma_start(
                        dst_rows[s],
                        hnew[:].rearrange("p (t d) -> p t d", d=D))
                else:
                    t2_rows = t2_own[L].rearrange(
                        "(s t p) d -> s p t d", s=NST, t=SPT, p=P)
                    nc.sync.dma_start(
                        t2_rows[s],
                        t2b[:].rearrange("p (t d) -> p t d", d=DP))
            if not last:
                nc.gpsimd.collective_compute(
                    "AllGather",
                    mybir.AluOpType.bypass,
                    replica_groups=[list(range(cfg.n_cores))],
                    ins=[t2_own[L][:]],
                    outs=[h_full[L][:]],
                )


def _make_cfg_full():
    return Cfg(n_nodes=100000, n_cores=8, npc_raw=12500, npc=12800,
               spt=5, nb=4, ctb=3)


def kernel(**inputs):
    cfg = _make_cfg_full()
    x = np.asarray(inputs["x"], np.float32)
    ei = np.asarray(inputs["edge_index"])
    Wgs, Wl = [], []
    for L, (a, b, c, bias) in enumerate(
            [("Wg1", "Wl1", "Ws1", "b1"), ("Wg2", "Wl2", "Ws2", "b2"),
             ("Wgo", "Wlo", "Wso", "bo")]):
        bv = np.asarray(inputs[bias], np.float32)
        assert np.all(bv == 0.0), "nonzero bias not supported by this build"
        Wgs.append((np.asarray(inputs[a], np.float32) +
                    np.asarray(inputs[c], np.float32)).astype(BF_NP))
        Wl.append(np.asarray(inputs[b], np.float32))

    per_core, perm_pos = prep_host(x, ei, Wl[0], cfg)

    nc = bacc.Bacc("TRN2", target_bir_lowering=False, debug=False,
                   enable_asserts=False, num_devices=cfg.n_cores)
    build_program(nc, cfg)
    nc.compile()

    in_maps = []
    for c in range(cfg.n_cores):
        m = dict(per_core[c])
        for L in range(3):
            m[f"w_gs{L}"] = Wgs[L]
        for L in range(2):
            m[f"w_ln{L}"] = Wl[L + 1].astype(BF_NP)
        in_maps.append(m)

    res = run_bass_kernel_spmd(
        nc, in_maps, core_ids=list(range(cfg.n_cores)),
        trace=bool(int(os.environ.get("GNN_TRACE", "0"))),
    )
    full = np.zeros((cfg.n_nodes, D), np.float32)
    for c in range(cfg.n_cores):
        lo = c * cfg.npc_raw
        hi = min((c + 1) * cfg.npc_raw, cfg.n_nodes)
        full[lo:hi] = res.results[c]["out_own"][perm_pos[lo:hi]]
    kernel.last_results = res
    return full.astype(np.float32)


# revision 8
# speedup vs baseline: 3.4104x; 2.6049x over previous
"""DEMONet 3-layer GNN message-passing kernel for 8x Trainium2 NeuronCores.

Math per layer (verified against reference; all nodes have deg > 0):
    nm   = segment_sum(h[dst], src) / deg
    out  = elu(h @ (Wg + Ws) + nm @ Wl + b)          (b == 0 here)

Key structure (v2):
  * Nodes row-partitioned across 8 cores (12.5k real + pad -> 12800 per
    core).  Edges live with their src node's core.
  * The gather table for layer L holds rows (h_{L-1} @ Wl_L) in bf16,
    padded to 128 cols (256B rows).  Folding Wl into the table lets the
    on-device segment-sum produce nm @ Wl directly, removing the per-tile
    transpose + second matmul.  Layer 0's table (x @ Wl1) is computed on
    the host; tables for layers 1,2 are produced on device and AllGathered.
  * Neighbour rows are fetched with gpsimd.dma_gather: ONE instruction per
    (supertile, bucket) gathers 1536 rows (vs. indirect_dma_start's 128),
    cutting SWDGE fixed overhead ~12x.  Indices are int16 relative to one
    of NB=4 node buckets (25600 rows < 32768).
  * Segment-sum on the TensorEngine: gathered rows X_ch [128e, 64] (bf16)
    are combined with a 0/1 indicator S_ch [128e, 128n] accumulating
    PSUM[128n, 64] over a tile's 12 chunks.  S is built once per
    (layer, supertile) with a single DVE is_equal.
  * h kept transposed [64, NPC] in bf16 in SBUF across layers (no DRAM
    roundtrip); per tile: g-term matmul, fused (SU*invdeg)+g, ELU chain,
    transpose to update hT, and the next-layer table row matmul.
"""

import os
import numpy as np
import ml_dtypes

import concourse.bass as bass
import concourse.bacc as bacc
import concourse.mybir as mybir
import concourse.tile as tile
from concourse.bass_utils import run_bass_kernel_spmd
from concourse.masks import make_identity

F32 = mybir.dt.float32
BF16 = mybir.dt.bfloat16
I32 = mybir.dt.int32
I16 = mybir.dt.int16
BF_NP = ml_dtypes.bfloat16

P = 128   # partitions / tile node count / chunk edge count
D = 64    # feature dim
DP = 128  # padded feature width of the gather table (256B rows)
GMAX = 8  # max chunks (of 128 rows) per dma_gather instruction (1024 rows)


class Cfg:
    def __init__(self, n_nodes, n_cores, npc_raw, npc, spt, nb, ctb,
                 n_layers=3):
        self.n_nodes = n_nodes
        self.n_cores = n_cores
        self.npc_raw = npc_raw
        self.npc = npc                  # padded nodes per core
        self.tpc = npc // P             # tiles per core
        self.spt = spt                  # tiles per supertile
        self.nst = self.tpc // spt
        self.nb = nb                    # index buckets (core-aligned)
        self.ctb = ctb                  # chunks per (tile, bucket)
        self.cpt = nb * ctb             # chunks per tile
        self.j = spt * self.cpt         # chunks per supertile
        self.ntot = n_cores * npc
        self.bs = self.ntot // nb       # bucket size (rows); must be < 32768
        assert self.bs <= 32768
        assert (n_cores * npc) % nb == 0 and npc * (n_cores // nb) == self.bs
        self.n_layers = n_layers


def _pack_core(sizes, tpc, cap):
    """Assign nodes (rows of `sizes` [n,nb]) to tpc tiles of 128 slots s.t.
    per-tile per-bucket sums <= cap.  Returns tile index per node."""
    n, nbk = sizes.shape
    order = np.argsort(-sizes.sum(1), kind="stable")
    rem = np.full((tpc, nbk), cap, np.int64)
    slots = np.full(tpc, P)
    assign = np.full(n, -1, np.int32)
    for i in order:
        s = sizes[i]
        cand = (slots > 0) & np.all(rem >= s, axis=1)
        if not cand.any():
            raise RuntimeError("node packing failed; increase ctb")
        scores = (rem - s).min(1).astype(np.float64) + 0.001 * slots
        scores[~cand] = -1e18
        t = int(np.argmax(scores))
        rem[t] -= s
        slots[t] -= 1
        assign[i] = t
    return assign


def prep_host(x, edge_index, Wl1, cfg: Cfg):
    N = cfg.n_nodes
    NC, NPC_RAW, NPC, TPC, SPT, NB, CTB = (
        cfg.n_cores, cfg.npc_raw, cfg.npc, cfg.tpc, cfg.spt, cfg.nb, cfg.ctb)
    NST, CPT, J = cfg.nst, cfg.cpt, cfg.j
    RCH = SPT * CTB
    BS = cfg.bs
    src = np.asarray(edge_index[0], dtype=np.int64)
    dst = np.asarray(edge_index[1], dtype=np.int64)
    E = src.shape[0]

    deg = np.bincount(src, minlength=N)
    if deg.min() == 0:
        raise NotImplementedError(
            "deg-0 nodes present; the simplified Wg+Ws fusion is invalid")
    inv_deg = (1.0 / deg).astype(np.float32)

    c_src = np.minimum(src // NPC_RAW, NC - 1)
    c_dst = np.minimum(dst // NPC_RAW, NC - 1)
    bucket = c_dst // (NC // NB)

    # per-node out-degree per bucket, then pack nodes into tiles
    nbcnt = np.zeros((N, NB), np.int32)
    np.add.at(nbcnt, (src, bucket), 1)
    perm_pos = np.zeros(N, np.int64)      # orig id -> position within core
    for c in range(NC):
        lo, hi = c * NPC_RAW, min((c + 1) * NPC_RAW, N)
        n_local = hi - lo
        assign = _pack_core(nbcnt[lo:hi], TPC, CTB * P)
        order_t = np.argsort(assign, kind="stable")
        within = np.arange(n_local) - np.searchsorted(
            assign[order_t], assign[order_t])
        pos = np.empty(n_local, np.int64)
        pos[order_t] = assign[order_t] * P + within
        perm_pos[lo:hi] = pos
    gpos = np.minimum(np.arange(N) // NPC_RAW, NC - 1) * NPC + perm_pos

    pdst = gpos[dst]                      # permuted global dst id
    lsrc_tile = (perm_pos[src] % P).astype(np.int16)
    tile_of_src = perm_pos[src] // P      # tile within core
    st_of_src = tile_of_src // SPT
    t_in_st = tile_of_src % SPT

    # slot assignment: group by (core, st, bucket, tile-in-st)
    key = ((c_src * NST + st_of_src) * NB + bucket) * SPT + t_in_st
    n_groups = NC * NST * NB * SPT
    counts = np.bincount(key, minlength=n_groups)
    assert counts.max() <= CTB * P, (counts.max(), CTB * P)
    order = np.argsort(key, kind="stable")
    starts = np.zeros(n_groups + 1, np.int64)
    np.cumsum(counts, out=starts[1:])
    q = np.arange(E) - starts[key[order]]     # position within group
    ks = key[order]
    g_c = ks // (NST * NB * SPT)
    g_st = (ks // (NB * SPT)) % NST
    g_b = (ks // SPT) % NB
    g_t = ks % SPT
    chunk = g_b * RCH + g_t * CTB + q // P   # chunk within supertile
    p = q % P

    lsrc_arr = np.full((NC, NST, P, J), 300, np.int16)
    lsrc_arr[g_c, g_st, p, chunk] = lsrc_tile[order]

    # bucket-relative int16 indices; pads point at bucket row 0 (their
    # contribution is killed by S == 0)
    rel = np.zeros((NC, NST, P, J), np.int16)
    rel[g_c, g_st, p, chunk] = (pdst[order] - g_b * BS).astype(np.int16)

    # dma_gather wrapped layout: idx i (-> partition i%128, chunk i//128 of
    # the output) is read from idxs[i%16, i//16]; replicate over 8 groups.
    W16 = RCH * P // 16                   # 96 idx columns per bucket
    blocks = rel.reshape(NC, NST, P, NB, RCH)
    flat = blocks.transpose(0, 1, 3, 4, 2).reshape(NC, NST, NB, RCH * P)
    w = flat.reshape(NC, NST, NB, W16, 16).transpose(0, 1, 2, 4, 3)
    wfull = np.broadcast_to(w[:, :, :, None, :, :],
                            (NC, NST, NB, 8, 16, W16))
    gidx16 = np.ascontiguousarray(
        wfull.reshape(NC, NST, NB, P, W16).transpose(0, 1, 3, 2, 4)
        .reshape(NC, NST, P, NB * W16))

    # padded per-core node arrays (in permuted order)
    x = np.asarray(x, dtype=np.float32)
    x_pad = np.zeros((cfg.ntot, D), np.float32)
    invdeg_pad = np.zeros(cfg.ntot, np.float32)
    x_pad[gpos] = x
    invdeg_pad[gpos] = inv_deg

    # layer-0 gather table: (x @ Wl1) in bf16, 256B rows
    t0 = (x_pad @ np.asarray(Wl1, np.float32)).astype(BF_NP)
    table0 = np.zeros((cfg.ntot, DP), BF_NP)
    table0[:, :D] = t0

    per_core = []
    for c in range(NC):
        xs = x_pad[c * NPC:(c + 1) * NPC]
        per_core.append(dict(
            x_ownT=np.ascontiguousarray(xs.T.astype(BF_NP)),      # [64, NPC]
            table0=table0,                                        # [NTOT, DP]
            gidx16=np.ascontiguousarray(gidx16[c]),               # [NST,128,NB*96]
            lsrc=np.ascontiguousarray(lsrc_arr[c]),               # [NST,128,J]
            invdegT=np.ascontiguousarray(
                invdeg_pad[c * NPC:(c + 1) * NPC].reshape(TPC, P).T),
        ))
    return per_core, perm_pos


def build_program(nc, cfg: Cfg, tc=None):
    NPC, NTOT, NST, SPT, NB, CTB, J, TPC = (
        cfg.npc, cfg.ntot, cfg.nst, cfg.spt, cfg.nb, cfg.ctb, cfg.j, cfg.tpc)
    NL = cfg.n_layers
    RCH = SPT * CTB
    W16 = RCH * P // 16

    x_ownT = nc.dram_tensor("x_ownT", [D, NPC], BF16, kind="ExternalInput")
    table0 = nc.dram_tensor("table0", [NTOT, DP], BF16, kind="ExternalInput")
    gidx16 = nc.dram_tensor("gidx16", [NST, P, NB * W16], I16,
                            kind="ExternalInput")
    lsrc = nc.dram_tensor("lsrc", [NST, P, J], I16, kind="ExternalInput")
    invdegT = nc.dram_tensor("invdegT", [P, TPC], F32, kind="ExternalInput")
    w_gs = [nc.dram_tensor(f"w_gs{L}", [D, D], BF16, kind="ExternalInput")
            for L in range(NL)]
    w_ln = [nc.dram_tensor(f"w_ln{L}", [D, D], BF16, kind="ExternalInput")
            for L in range(NL - 1)]   # Wl of layer L+1
    out_own = nc.dram_tensor("out_own", [NPC, D], F32, kind="ExternalOutput")

    t2_own = [nc.dram_tensor(f"t2_own{L}", [NPC, DP], BF16, kind="Internal")
              for L in range(NL - 1)]
    h_full = [nc.dram_tensor(f"h_full{L}", [NTOT, DP], BF16, kind="Internal",
                             addr_space="Shared" if cfg.n_cores > 4 else "Local")
              for L in range(NL - 1)]

    own_ctx = tc is None
    if own_ctx:
        tc = tile.TileContext(nc)
        tc.__enter__()
    try:
        _emit(nc, tc, cfg, locals())
    finally:
        if own_ctx:
            tc.__exit__(None, None, None)
    return nc


def _emit(nc, tc, cfg: Cfg, T):
    NPC, NTOT, NST, SPT, NB, CTB, J, TPC, NL = (
        cfg.npc, cfg.ntot, cfg.nst, cfg.spt, cfg.nb, cfg.ctb, cfg.j, cfg.tpc,
        cfg.n_layers)
    x_ownT, table0, gidx16, lsrc, invdegT = (
        T["x_ownT"], T["table0"], T["gidx16"], T["lsrc"], T["invdegT"])
    w_gs, w_ln, out_own = T["w_gs"], T["w_ln"], T["out_own"]
    t2_own, h_full = T["t2_own"], T["h_full"]
    RCH = SPT * CTB               # chunks per bucket region
    BS = cfg.bs
    W16 = RCH * P // 16

    with (
        tc.tile_pool(name="const", bufs=1) as constp,
        tc.tile_pool(name="io", bufs=3) as iop,
        tc.tile_pool(name="big", bufs=2) as bigp,
        tc.tile_pool(name="small", bufs=4) as smallp,
        tc.tile_pool(name="psA", bufs=2, space="PSUM") as psA,
        tc.tile_pool(name="psB", bufs=2, space="PSUM") as psB,
        tc.tile_pool(name="psC", bufs=2, space="PSUM") as psC,
        tc.tile_pool(name="psD", bufs=2, space="PSUM") as psD,
    ):
        ident = constp.tile([P, P], BF16, name="ident")
        make_identity(nc, ident[:])
        iota16 = constp.tile([P, P], I16, name="iota16")
        nc.gpsimd.iota(iota16[:], pattern=[[1, P]], base=0, channel_multiplier=0)
        invdeg_sb = constp.tile([P, TPC], F32, name="invdeg_sb")
        nc.sync.dma_start(invdeg_sb[:], invdegT[:])
        wgs_sb, wln_sb = [], []
        for L in range(NL):
            wg_t = constp.tile([D, D], BF16, name=f"wgs_sb{L}")
            nc.sync.dma_start(wg_t[:], w_gs[L][:])
            wgs_sb.append(wg_t)
        for L in range(NL - 1):
            wl_t = constp.tile([D, D], BF16, name=f"wln_sb{L}")
            nc.sync.dma_start(wl_t[:], w_ln[L][:])
            wln_sb.append(wl_t)
        hT = [constp.tile([D, NPC], BF16, name=f"hT{i}") for i in range(2)]
        nc.sync.dma_start(hT[0][:], x_ownT[:])

        for L in range(NL):
            table = table0 if L == 0 else h_full[L - 1]
            hT_in, hT_out = hT[L % 2], hT[(L + 1) % 2]
            last = L == NL - 1
            for s in range(NST):
                idx_t = iop.tile([P, NB * W16], I16, tag="idx",
                                 name=f"idx_{L}_{s}")
                nc.sync.dma_start(idx_t[:], gidx16[s, :, :])
                lsrc_t = iop.tile([P, J], I16, tag="lsrc", name=f"lsr_{L}_{s}")
                nc.sync.dma_start(lsrc_t[:], lsrc[s, :, :])

                X = bigp.tile([P, J * DP], BF16, tag="X", name=f"X_{L}_{s}")
                X3 = X[:].rearrange("p (c e) -> p c e", e=DP)
                # HW limit: <= 1024 gathered rows (8 chunks) per dma_gather.
                # Each queue runs on its own Q7 core pair (cpu_id/2 ==
                # queue_num in the ucode), so spreading over 4 queues
                # parallelizes descriptor generation 4x.
                gq = 0
                for b in range(NB):
                    for c0 in range(0, RCH, GMAX):
                        c1 = min(c0 + GMAX, RCH)
                        nidx = (c1 - c0) * P
                        nc.gpsimd.dma_gather(
                            out_ap=X3[:, b * RCH + c0:b * RCH + c1, :],
                            in_ap=table[b * BS:(b + 1) * BS, :],
                            idxs_ap=idx_t[:, b * W16 + c0 * (P // 16):
                                          b * W16 + c1 * (P // 16)],
                            num_idxs=nidx,
                            num_idxs_reg=nidx,
                            elem_size=DP,
                            queue_num=gq % 4,
                        )
                        gq += 1

                S = bigp.tile([P, J * P], BF16, tag="S", name=f"S_{L}_{s}")
                nc.vector.tensor_tensor(
                    out=S[:].rearrange("p (j i) -> p j i", i=P),
                    in0=lsrc_t[:, :, None].to_broadcast([P, J, P]),
                    in1=iota16[:, None, :].to_broadcast([P, J, P]),
                    op=mybir.AluOpType.is_equal,
                )

                if last:
                    hnew = bigp.tile([P, SPT * D], F32, tag="hnew",
                                     name=f"hn_{L}_{s}")
                else:
                    hnb = bigp.tile([P, SPT * D], BF16, tag="hnb",
                                    name=f"hb_{L}_{s}")
                    t2b = bigp.tile([P, SPT * DP], BF16, tag="t2b",
                                    name=f"t2_{L}_{s}")

                for t in range(SPT):
                    g_t = s * SPT + t
                    chunks = [b * RCH + t * CTB + k
                              for b in range(NB) for k in range(CTB)]
                    SU = psA.tile([P, D], F32, tag="SU", name=f"SU_{L}_{s}_{t}")
                    for ci, ch in enumerate(chunks):
                        nc.tensor.matmul(
                            SU[:],
                            lhsT=S[:, ch * P:(ch + 1) * P],
                            rhs=X3[:, ch, 0:D],
                            start=(ci == 0), stop=(ci == len(chunks) - 1),
                        )
                    gP = psC.tile([P, D], F32, tag="gP", name=f"gP_{L}_{s}_{t}")
                    nc.tensor.matmul(
                        gP[:], lhsT=hT_in[:, g_t * P:(g_t + 1) * P],
                        rhs=wgs_sb[L][:], start=True, stop=True)
                    # pre = SU * invdeg + g  (two ops: only one PSUM input
                    # allowed per DVE instruction)
                    e_sb = smallp.tile([P, D], F32, tag="e",
                                       name=f"e_{L}_{s}_{t}")
                    nc.vector.tensor_scalar_mul(
                        e_sb[:], SU[:], invdeg_sb[:, g_t:g_t + 1])
                    pre = smallp.tile([P, D], F32, tag="pre",
                                      name=f"pr_{L}_{s}_{t}")
                    nc.vector.tensor_add(pre[:], e_sb[:], gP[:])
                    # ELU: out = (max(x,0)-1) + exp(min(x,0))
                    lo = smallp.tile([P, D], F32, tag="lo", name=f"lo_{L}_{s}_{t}")
                    nc.vector.tensor_scalar_min(lo[:], pre[:], 0.0)
                    ex = smallp.tile([P, D], F32, tag="ex", name=f"ex_{L}_{s}_{t}")
                    nc.scalar.activation(ex[:], lo[:],
                                         mybir.ActivationFunctionType.Exp)
                    hi1 = smallp.tile([P, D], F32, tag="hi1",
                                      name=f"hi_{L}_{s}_{t}")
                    nc.vector.tensor_scalar(
                        hi1[:], pre[:], 0.0, 1.0,
                        op0=mybir.AluOpType.max, op1=mybir.AluOpType.subtract)
                    if last:
                        nc.vector.tensor_add(
                            hnew[:, t * D:(t + 1) * D], ex[:], hi1[:])
                    else:
                        nc.vector.tensor_add(
                            hnb[:, t * D:(t + 1) * D], ex[:], hi1[:])
                        hTP = psD.tile([D, P], BF16, tag="hTP",
                                       name=f"hTP_{L}_{s}_{t}")
                        nc.tensor.transpose(
                            hTP[:], hnb[:, t * D:(t + 1) * D], ident[:])
                        nc.vector.tensor_copy(
                            hT_out[:, g_t * P:(g_t + 1) * P], hTP[:])
                        t2P = psB.tile([P, D], F32, tag="t2P",
                                       name=f"t2P_{L}_{s}_{t}")
                        nc.tensor.matmul(
                            t2P[:], lhsT=hT_out[:, g_t * P:(g_t + 1) * P],
                            rhs=wln_sb[L][:], start=True, stop=True)
                        nc.vector.tensor_copy(
                            t2b[:, t * DP:t * DP + D], t2P[:])

                if last:
                    dst_rows = out_own.rearrange(
                        "(s t p) d -> s p t d", s=NST, t=SPT, p=P)
                    nc.sync.dma_start(
                        dst_rows[s],
                        hnew[:].rearrange("p (t d) -> p t d", d=D))
                else:
                    t2_rows = t2_own[L].rearrange(
                        "(s t p) d -> s p t d", s=NST, t=SPT, p=P)
                    nc.sync.dma_start(
                        t2_rows[s],
                        t2b[:].rearrange("p (t d) -> p t d", d=DP))
            if not last:
                nc.gpsimd.collective_compute(
                    "AllGather",
                    mybir.AluOpType.bypass,
                    replica_groups=[list(range(cfg.n_cores))],
                    ins=[t2_own[L][:]],
                    outs=[h_full[L][:]],
                )


def _make_cfg_full():
    return Cfg(n_nodes=100000, n_cores=8, npc_raw=12500, npc=12800,
               spt=5, nb=4, ctb=3)


def kernel(**inputs):
    cfg = _make_cfg_full()
    x = np.asarray(inputs["x"], np.float32)
    ei = np.asarray(inputs["edge_index"])
    Wgs, Wl = [], []
    for L, (a, b, c, bias) in enumerate(
            [("Wg1", "Wl1", "Ws1", "b1"), ("Wg2", "Wl2", "Ws2", "b2"),
             ("Wgo", "Wlo", "Wso", "bo")]):
        bv = np.asarray(inputs[bias], np.float32)
        assert np.all(bv == 0.0), "nonzero bias not supported by this build"
        Wgs.append((np.asarray(inputs[a], np.float32) +
                    np.asarray(inputs[c], np.float32)).astype(BF_NP))
        Wl.append(np.asarray(inputs[b], np.float32))

    per_core, perm_pos = prep_host(x, ei, Wl[0], cfg)

    nc = bacc.Bacc("TRN2", target_bir_lowering=False, debug=False,
                   enable_asserts=False, num_devices=cfg.n_cores,
                   num_swdge_queues=4)
    build_program(nc, cfg)
    nc.compile()

    in_maps = []
    for c in range(cfg.n_cores):
        m = dict(per_core[c])
        for L in range(3):
            m[f"w_gs{L}"] = Wgs[L]
        for L in range(2):
            m[f"w_ln{L}"] = Wl[L + 1].astype(BF_NP)
        in_maps.append(m)

    res = run_bass_kernel_spmd(
        nc, in_maps, core_ids=list(range(cfg.n_cores)),
        trace=bool(int(os.environ.get("GNN_TRACE", "0"))),
    )
    full = np.zeros((cfg.n_nodes, D), np.float32)
    for c in range(cfg.n_cores):
        lo = c * cfg.npc_raw
        hi = min((c + 1) * cfg.npc_raw, cfg.n_nodes)
        full[lo:hi] = res.results[c]["out_own"][perm_pos[lo:hi]]
    kernel.last_results = res
    return full.astype(np.float32)


# revision 12
# speedup vs baseline: 3.5577x; 1.0432x over previous
"""DEMONet 3-layer GNN message-passing kernel for 8x Trainium2 NeuronCores.

Math per layer (verified against reference; all nodes have deg > 0):
    nm   = segment_sum(h[dst], src) / deg
    out  = elu(h @ (Wg + Ws) + nm @ Wl + b)          (b == 0 here)

Key structure (v2):
  * Nodes row-partitioned across 8 cores (12.5k real + pad -> 12800 per
    core).  Edges live with their src node's core.
  * The gather table for layer L holds rows (h_{L-1} @ Wl_L) in bf16,
    padded to 128 cols (256B rows).  Folding Wl into the table lets the
    on-device segment-sum produce nm @ Wl directly, removing the per-tile
    transpose + second matmul.  Layer 0's table (x @ Wl1) is computed on
    the host; tables for layers 1,2 are produced on device and AllGathered.
  * Neighbour rows are fetched with gpsimd.dma_gather: ONE instruction per
    (supertile, bucket) gathers 1536 rows (vs. indirect_dma_start's 128),
    cutting SWDGE fixed overhead ~12x.  Indices are int16 relative to one
    of NB=4 node buckets (25600 rows < 32768).
  * Segment-sum on the TensorEngine: gathered rows X_ch [128e, 64] (bf16)
    are combined with a 0/1 indicator S_ch [128e, 128n] accumulating
    PSUM[128n, 64] over a tile's 12 chunks.  S is built once per
    (layer, supertile) with a single DVE is_equal.
  * h kept transposed [64, NPC] in bf16 in SBUF across layers (no DRAM
    roundtrip); per tile: g-term matmul, fused (SU*invdeg)+g, ELU chain,
    transpose to update hT, and the next-layer table row matmul.
"""

import os
import numpy as np
import ml_dtypes

import concourse.bass as bass
import concourse.bacc as bacc
import concourse.mybir as mybir
import concourse.tile as tile
from concourse.bass_utils import run_bass_kernel_spmd
from concourse.masks import make_identity

F32 = mybir.dt.float32
BF16 = mybir.dt.bfloat16
I32 = mybir.dt.int32
I16 = mybir.dt.int16
BF_NP = ml_dtypes.bfloat16

P = 128   # partitions / tile node count / chunk edge count
D = 64    # feature dim
DP = 128  # padded feature width of the gather table (256B rows)
GMAX = 8  # max chunks (of 128 rows) per dma_gather instruction (1024 rows)


class Cfg:
    def __init__(self, n_nodes, n_cores, npc_raw, npc, spt, nb, ctb,
                 n_layers=3):
        self.n_nodes = n_nodes
        self.n_cores = n_cores
        self.npc_raw = npc_raw
        self.npc = npc                  # padded nodes per core
        self.tpc = npc // P             # tiles per core
        self.spt = spt                  # tiles per supertile
        self.nst = self.tpc // spt
        self.nb = nb                    # index buckets (core-aligned)
        self.ctb = ctb                  # chunks per (tile, bucket)
        self.cpt = nb * ctb             # chunks per tile
        self.j = spt * self.cpt         # chunks per supertile
        self.ntot = n_cores * npc
        self.bs = self.ntot // nb       # bucket size (rows); must be < 32768
        assert self.bs <= 32768
        assert (n_cores * npc) % nb == 0 and npc * (n_cores // nb) == self.bs
        self.n_layers = n_layers


def _pack_core(sizes, tpc, cap):
    """Assign nodes (rows of `sizes` [n,nb]) to tpc tiles of 128 slots s.t.
    per-tile per-bucket sums <= cap.  Returns tile index per node."""
    n, nbk = sizes.shape
    order = np.argsort(-sizes.sum(1), kind="stable")
    rem = np.full((tpc, nbk), cap, np.int64)
    slots = np.full(tpc, P)
    assign = np.full(n, -1, np.int32)
    for i in order:
        s = sizes[i]
        cand = (slots > 0) & np.all(rem >= s, axis=1)
        if not cand.any():
            raise RuntimeError("node packing failed; increase ctb")
        scores = (rem - s).min(1).astype(np.float64) + 0.001 * slots
        scores[~cand] = -1e18
        t = int(np.argmax(scores))
        rem[t] -= s
        slots[t] -= 1
        assign[i] = t
    return assign


def prep_host(x, edge_index, Wl1, cfg: Cfg):
    N = cfg.n_nodes
    NC, NPC_RAW, NPC, TPC, SPT, NB, CTB = (
        cfg.n_cores, cfg.npc_raw, cfg.npc, cfg.tpc, cfg.spt, cfg.nb, cfg.ctb)
    NST, CPT, J = cfg.nst, cfg.cpt, cfg.j
    RCH = SPT * CTB
    BS = cfg.bs
    src = np.asarray(edge_index[0], dtype=np.int64)
    dst = np.asarray(edge_index[1], dtype=np.int64)
    E = src.shape[0]

    deg = np.bincount(src, minlength=N)
    if deg.min() == 0:
        raise NotImplementedError(
            "deg-0 nodes present; the simplified Wg+Ws fusion is invalid")
    inv_deg = (1.0 / deg).astype(np.float32)

    c_src = np.minimum(src // NPC_RAW, NC - 1)
    c_dst = np.minimum(dst // NPC_RAW, NC - 1)
    bucket = c_dst // (NC // NB)

    # per-node out-degree per bucket, then pack nodes into tiles
    nbcnt = np.zeros((N, NB), np.int32)
    np.add.at(nbcnt, (src, bucket), 1)
    perm_pos = np.zeros(N, np.int64)      # orig id -> position within core
    for c in range(NC):
        lo, hi = c * NPC_RAW, min((c + 1) * NPC_RAW, N)
        n_local = hi - lo
        assign = _pack_core(nbcnt[lo:hi], TPC, CTB * P)
        order_t = np.argsort(assign, kind="stable")
        within = np.arange(n_local) - np.searchsorted(
            assign[order_t], assign[order_t])
        pos = np.empty(n_local, np.int64)
        pos[order_t] = assign[order_t] * P + within
        perm_pos[lo:hi] = pos
    gpos = np.minimum(np.arange(N) // NPC_RAW, NC - 1) * NPC + perm_pos

    pdst = gpos[dst]                      # permuted global dst id
    lsrc_tile = (perm_pos[src] % P).astype(np.int16)
    tile_of_src = perm_pos[src] // P      # tile within core
    st_of_src = tile_of_src // SPT
    t_in_st = tile_of_src % SPT

    # slot assignment: group by (core, st, bucket, tile-in-st)
    key = ((c_src * NST + st_of_src) * NB + bucket) * SPT + t_in_st
    n_groups = NC * NST * NB * SPT
    counts = np.bincount(key, minlength=n_groups)
    assert counts.max() <= CTB * P, (counts.max(), CTB * P)
    order = np.argsort(key, kind="stable")
    starts = np.zeros(n_groups + 1, np.int64)
    np.cumsum(counts, out=starts[1:])
    q = np.arange(E) - starts[key[order]]     # position within group
    ks = key[order]
    g_c = ks // (NST * NB * SPT)
    g_st = (ks // (NB * SPT)) % NST
    g_b = (ks // SPT) % NB
    g_t = ks % SPT
    chunk = g_b * RCH + g_t * CTB + q // P   # chunk within supertile
    p = q % P

    lsrc_arr = np.full((NC, NST, P, J), 300, np.int16)
    lsrc_arr[g_c, g_st, p, chunk] = lsrc_tile[order]

    # bucket-relative int16 indices; pads point at bucket row 0 (their
    # contribution is killed by S == 0)
    rel = np.zeros((NC, NST, P, J), np.int16)
    rel[g_c, g_st, p, chunk] = (pdst[order] - g_b * BS).astype(np.int16)

    # dma_gather wrapped layout: idx i (-> partition i%128, chunk i//128 of
    # the output) is read from idxs[i%16, i//16]; replicate over 8 groups.
    W16 = RCH * P // 16                   # 96 idx columns per bucket
    blocks = rel.reshape(NC, NST, P, NB, RCH)
    flat = blocks.transpose(0, 1, 3, 4, 2).reshape(NC, NST, NB, RCH * P)
    w = flat.reshape(NC, NST, NB, W16, 16).transpose(0, 1, 2, 4, 3)
    wfull = np.broadcast_to(w[:, :, :, None, :, :],
                            (NC, NST, NB, 8, 16, W16))
    gidx16 = np.ascontiguousarray(
        wfull.reshape(NC, NST, NB, P, W16).transpose(0, 1, 3, 2, 4)
        .reshape(NC, NST, P, NB * W16))

    # padded per-core node arrays (in permuted order)
    x = np.asarray(x, dtype=np.float32)
    x_pad = np.zeros((cfg.ntot, D), np.float32)
    invdeg_pad = np.zeros(cfg.ntot, np.float32)
    x_pad[gpos] = x
    invdeg_pad[gpos] = inv_deg

    # layer-0 gather table: (x @ Wl1) in bf16, 256B rows
    t0 = (x_pad @ np.asarray(Wl1, np.float32)).astype(BF_NP)
    table0 = np.zeros((cfg.ntot, DP), BF_NP)
    table0[:, :D] = t0

    per_core = []
    for c in range(NC):
        xs = x_pad[c * NPC:(c + 1) * NPC]
        per_core.append(dict(
            x_ownT=np.ascontiguousarray(xs.T.astype(BF_NP)),      # [64, NPC]
            table0=table0,                                        # [NTOT, DP]
            gidx16=np.ascontiguousarray(gidx16[c]),               # [NST,128,NB*96]
            lsrc=np.ascontiguousarray(lsrc_arr[c]),               # [NST,128,J]
            invdegT=np.ascontiguousarray(
                invdeg_pad[c * NPC:(c + 1) * NPC].reshape(TPC, P).T),
        ))
    return per_core, perm_pos


def build_program(nc, cfg: Cfg, tc=None):
    NPC, NTOT, NST, SPT, NB, CTB, J, TPC = (
        cfg.npc, cfg.ntot, cfg.nst, cfg.spt, cfg.nb, cfg.ctb, cfg.j, cfg.tpc)
    NL = cfg.n_layers
    RCH = SPT * CTB
    W16 = RCH * P // 16

    x_ownT = nc.dram_tensor("x_ownT", [D, NPC], BF16, kind="ExternalInput")
    table0 = nc.dram_tensor("table0", [NTOT, DP], BF16, kind="ExternalInput")
    gidx16 = nc.dram_tensor("gidx16", [NST, P, NB * W16], I16,
                            kind="ExternalInput")
    lsrc = nc.dram_tensor("lsrc", [NST, P, J], I16, kind="ExternalInput")
    invdegT = nc.dram_tensor("invdegT", [P, TPC], F32, kind="ExternalInput")
    w_gs = [nc.dram_tensor(f"w_gs{L}", [D, D], BF16, kind="ExternalInput")
            for L in range(NL)]
    w_ln = [nc.dram_tensor(f"w_ln{L}", [D, D], BF16, kind="ExternalInput")
            for L in range(NL - 1)]   # Wl of layer L+1
    out_own = nc.dram_tensor("out_own", [NPC, D], F32, kind="ExternalOutput")

    t2_own = [nc.dram_tensor(f"t2_own{L}", [NPC, DP], BF16, kind="Internal")
              for L in range(NL - 1)]
    s_cache = nc.dram_tensor("s_cache", [NST, P, SPT * NB * CTB * P], BF16,
                             kind="Internal")
    h_full = [nc.dram_tensor(f"h_full{L}", [NTOT, DP], BF16, kind="Internal",
                             addr_space="Shared" if cfg.n_cores > 4 else "Local")
              for L in range(NL - 1)]

    own_ctx = tc is None
    if own_ctx:
        tc = tile.TileContext(nc)
        tc.__enter__()
    try:
        _emit(nc, tc, cfg, locals())
    finally:
        if own_ctx:
            tc.__exit__(None, None, None)
    return nc


def _emit(nc, tc, cfg: Cfg, T):
    NPC, NTOT, NST, SPT, NB, CTB, J, TPC, NL = (
        cfg.npc, cfg.ntot, cfg.nst, cfg.spt, cfg.nb, cfg.ctb, cfg.j, cfg.tpc,
        cfg.n_layers)
    x_ownT, table0, gidx16, lsrc, invdegT = (
        T["x_ownT"], T["table0"], T["gidx16"], T["lsrc"], T["invdegT"])
    w_gs, w_ln, out_own = T["w_gs"], T["w_ln"], T["out_own"]
    t2_own, h_full, s_cache = T["t2_own"], T["h_full"], T["s_cache"]
    RCH = SPT * CTB               # chunks per bucket region
    BS = cfg.bs
    W16 = RCH * P // 16

    with (
        tc.tile_pool(name="const", bufs=1) as constp,
        tc.tile_pool(name="io", bufs=3) as iop,
        tc.tile_pool(name="big", bufs=2) as bigp,
        tc.tile_pool(name="small", bufs=4) as smallp,
        tc.tile_pool(name="psA", bufs=2, space="PSUM") as psA,
        tc.tile_pool(name="psB", bufs=2, space="PSUM") as psB,
        tc.tile_pool(name="psC", bufs=2, space="PSUM") as psC,
        tc.tile_pool(name="psD", bufs=2, space="PSUM") as psD,
    ):
        ident = constp.tile([P, P], BF16, name="ident")
        make_identity(nc, ident[:])
        iota16 = constp.tile([P, P], I16, name="iota16")
        nc.gpsimd.iota(iota16[:], pattern=[[1, P]], base=0, channel_multiplier=0)
        invdeg_sb = constp.tile([P, TPC], F32, name="invdeg_sb")
        nc.sync.dma_start(invdeg_sb[:], invdegT[:])
        wgs_sb, wln_sb = [], []
        for L in range(NL):
            wg_t = constp.tile([D, D], BF16, name=f"wgs_sb{L}")
            nc.sync.dma_start(wg_t[:], w_gs[L][:])
            wgs_sb.append(wg_t)
        for L in range(NL - 1):
            wl_t = constp.tile([D, D], BF16, name=f"wln_sb{L}")
            nc.sync.dma_start(wl_t[:], w_ln[L][:])
            wln_sb.append(wl_t)
        hT = [constp.tile([D, NPC], BF16, name=f"hT{i}") for i in range(2)]
        nc.sync.dma_start(hT[0][:], x_ownT[:])

        for L in range(NL):
            table = table0 if L == 0 else h_full[L - 1]
            hT_in, hT_out = hT[L % 2], hT[(L + 1) % 2]
            last = L == NL - 1
            for s in range(NST):
                idx_t = iop.tile([P, NB * W16], I16, tag="idx",
                                 name=f"idx_{L}_{s}")
                nc.sync.dma_start(idx_t[:], gidx16[s, :, :])

                X = bigp.tile([P, J * DP], BF16, tag="X", name=f"X_{L}_{s}")
                X3 = X[:].rearrange("p (c e) -> p c e", e=DP)
                # HW limit: <= 1024 gathered rows (8 chunks) per dma_gather.
                # Each queue runs on its own Q7 core pair (cpu_id/2 ==
                # queue_num in the ucode), so spreading over 4 queues
                # parallelizes descriptor generation 4x.
                gq = 0
                for b in range(NB):
                    for c0 in range(0, RCH, GMAX):
                        c1 = min(c0 + GMAX, RCH)
                        nidx = (c1 - c0) * P
                        nc.gpsimd.dma_gather(
                            out_ap=X3[:, b * RCH + c0:b * RCH + c1, :],
                            in_ap=table[b * BS:(b + 1) * BS, :],
                            idxs_ap=idx_t[:, b * W16 + c0 * (P // 16):
                                          b * W16 + c1 * (P // 16)],
                            num_idxs=nidx,
                            num_idxs_reg=nidx,
                            elem_size=DP,
                            queue_num=gq % 4,
                        )
                        gq += 1

                S = bigp.tile([P, J * P], BF16, tag="S", name=f"S_{L}_{s}")
                if L == 0:
                    # S depends only on the graph: build once, cache in DRAM
                    lsrc_t = iop.tile([P, J], I16, tag="lsrc",
                                      name=f"lsr_{L}_{s}")
                    nc.sync.dma_start(lsrc_t[:], lsrc[s, :, :])
                    nc.vector.tensor_tensor(
                        out=S[:].rearrange("p (j i) -> p j i", i=P),
                        in0=lsrc_t[:, :, None].to_broadcast([P, J, P]),
                        in1=iota16[:, None, :].to_broadcast([P, J, P]),
                        op=mybir.AluOpType.is_equal,
                    )
                    nc.sync.dma_start(s_cache[s, :, :], S[:])
                else:
                    nc.sync.dma_start(S[:], s_cache[s, :, :])

                pre_st = bigp.tile([P, SPT * D], F32, tag="pre",
                                   name=f"pre_{L}_{s}")
                if last:
                    hnew = bigp.tile([P, SPT * D], F32, tag="hnew",
                                     name=f"hn_{L}_{s}")
                else:
                    hnb = bigp.tile([P, SPT * D], BF16, tag="hnb",
                                    name=f"hb_{L}_{s}")
                    t2b = bigp.tile([P, SPT * DP], BF16, tag="t2b",
                                    name=f"t2_{L}_{s}")

                for t in range(SPT):
                    g_t = s * SPT + t
                    chunks = [b * RCH + t * CTB + k
                              for b in range(NB) for k in range(CTB)]
                    SU = psA.tile([P, D], F32, tag="SU", name=f"SU_{L}_{s}_{t}")
                    for ci, ch in enumerate(chunks):
                        nc.tensor.matmul(
                            SU[:],
                            lhsT=S[:, ch * P:(ch + 1) * P],
                            rhs=X3[:, ch, 0:D],
                            start=(ci == 0), stop=(ci == len(chunks) - 1),
                        )
                    gP = psC.tile([P, D], F32, tag="gP", name=f"gP_{L}_{s}_{t}")
                    nc.tensor.matmul(
                        gP[:], lhsT=hT_in[:, g_t * P:(g_t + 1) * P],
                        rhs=wgs_sb[L][:], start=True, stop=True)
                    # pre = SU * invdeg + g  (two ops: only one PSUM input
                    # allowed per DVE instruction)
                    e_sb = smallp.tile([P, D], F32, tag="e",
                                       name=f"e_{L}_{s}_{t}")
                    nc.vector.tensor_scalar_mul(
                        e_sb[:], SU[:], invdeg_sb[:, g_t:g_t + 1])
                    nc.vector.tensor_add(
                        pre_st[:, t * D:(t + 1) * D], e_sb[:], gP[:])

                # batched ELU over the whole supertile:
                # out = (max(x,0)-1) + exp(min(x,0))
                lo = bigp.tile([P, SPT * D], F32, tag="lo", name=f"lo_{L}_{s}")
                nc.vector.tensor_scalar_min(lo[:], pre_st[:], 0.0)
                ex = bigp.tile([P, SPT * D], F32, tag="ex", name=f"ex_{L}_{s}")
                nc.scalar.activation(ex[:], lo[:],
                                     mybir.ActivationFunctionType.Exp)
                hi1 = bigp.tile([P, SPT * D], F32, tag="hi1",
                                name=f"hi_{L}_{s}")
                nc.vector.tensor_scalar(
                    hi1[:], pre_st[:], 0.0, 1.0,
                    op0=mybir.AluOpType.max, op1=mybir.AluOpType.subtract)
                nc.vector.tensor_add(hnew[:] if last else hnb[:],
                                     ex[:], hi1[:])

                if not last:
                    for t in range(SPT):
                        g_t = s * SPT + t
                        hTP = psD.tile([D, P], BF16, tag="hTP",
                                       name=f"hTP_{L}_{s}_{t}")
                        nc.tensor.transpose(
                            hTP[:], hnb[:, t * D:(t + 1) * D], ident[:])
                        nc.vector.tensor_copy(
                            hT_out[:, g_t * P:(g_t + 1) * P], hTP[:])
                        t2P = psB.tile([P, D], F32, tag="t2P",
                                       name=f"t2P_{L}_{s}_{t}")
                        nc.tensor.matmul(
                            t2P[:], lhsT=hT_out[:, g_t * P:(g_t + 1) * P],
                            rhs=wln_sb[L][:], start=True, stop=True)
                        nc.vector.tensor_copy(
                            t2b[:, t * DP:t * DP + D], t2P[:])

                if last:
                    dst_rows = out_own.rearrange(
                        "(s t p) d -> s p t d", s=NST, t=SPT, p=P)
                    nc.sync.dma_start(
                        dst_rows[s],
                        hnew[:].rearrange("p (t d) -> p t d", d=D))
                else:
                    t2_rows = t2_own[L].rearrange(
                        "(s t p) d -> s p t d", s=NST, t=SPT, p=P)
                    nc.sync.dma_start(
                        t2_rows[s],
                        t2b[:].rearrange("p (t d) -> p t d", d=DP))
            if not last:
                nc.gpsimd.collective_compute(
                    "AllGather",
                    mybir.AluOpType.bypass,
                    replica_groups=[list(range(cfg.n_cores))],
                    ins=[t2_own[L][:]],
                    outs=[h_full[L][:]],
                )


def _make_cfg_full():
    return Cfg(n_nodes=100000, n_cores=8, npc_raw=12500, npc=12800,
               spt=5, nb=4, ctb=3)


def kernel(**inputs):
    cfg = _make_cfg_full()
    x = np.asarray(inputs["x"], np.float32)
    ei = np.asarray(inputs["edge_index"])
    Wgs, Wl = [], []
    for L, (a, b, c, bias) in enumerate(
            [("Wg1", "Wl1", "Ws1", "b1"), ("Wg2", "Wl2", "Ws2", "b2"),
             ("Wgo", "Wlo", "Wso", "bo")]):
        bv = np.asarray(inputs[bias], np.float32)
        assert np.all(bv == 0.0), "nonzero bias not supported by this build"
        Wgs.append((np.asarray(inputs[a], np.float32) +
                    np.asarray(inputs[c], np.float32)).astype(BF_NP))
        Wl.append(np.asarray(inputs[b], np.float32))

    per_core, perm_pos = prep_host(x, ei, Wl[0], cfg)

    nc = bacc.Bacc("TRN2", target_bir_lowering=False, debug=False,
                   enable_asserts=False, num_devices=cfg.n_cores,
                   num_swdge_queues=4)
    build_program(nc, cfg)
    nc.compile()

    in_maps = []
    for c in range(cfg.n_cores):
        m = dict(per_core[c])
        for L in range(3):
            m[f"w_gs{L}"] = Wgs[L]
        for L in range(2):
            m[f"w_ln{L}"] = Wl[L + 1].astype(BF_NP)
        in_maps.append(m)

    res = run_bass_kernel_spmd(
        nc, in_maps, core_ids=list(range(cfg.n_cores)),
        trace=bool(int(os.environ.get("GNN_TRACE", "0"))),
    )
    full = np.zeros((cfg.n_nodes, D), np.float32)
    for c in range(cfg.n_cores):
        lo = c * cfg.npc_raw
        hi = min((c + 1) * cfg.npc_raw, cfg.n_nodes)
        full[lo:hi] = res.results[c]["out_own"][perm_pos[lo:hi]]
    kernel.last_results = res
    return full.astype(np.float32)


# revision 21
# speedup vs baseline: 3.8514x; 1.0825x over previous
"""DEMONet 3-layer GNN message-passing kernel for 8x Trainium2 NeuronCores.

Math per layer (verified against reference; all nodes have deg > 0):
    nm   = segment_sum(h[dst], src) / deg
    out  = elu(h @ (Wg + Ws) + nm @ Wl + b)          (b == 0 here)

Key structure (v2):
  * Nodes row-partitioned across 8 cores (12.5k real + pad -> 12800 per
    core).  Edges live with their src node's core.
  * The gather table for layer L holds rows (h_{L-1} @ Wl_L) in bf16,
    padded to 128 cols (256B rows).  Folding Wl into the table lets the
    on-device segment-sum produce nm @ Wl directly, removing the per-tile
    transpose + second matmul.  Layer 0's table (x @ Wl1) is computed on
    the host; tables for layers 1,2 are produced on device and AllGathered.
  * Neighbour rows are fetched with gpsimd.dma_gather: ONE instruction per
    (supertile, bucket) gathers 1536 rows (vs. indirect_dma_start's 128),
    cutting SWDGE fixed overhead ~12x.  Indices are int16 relative to one
    of NB=4 node buckets (25600 rows < 32768).
  * Segment-sum on the TensorEngine: gathered rows X_ch [128e, 64] (bf16)
    are combined with a 0/1 indicator S_ch [128e, 128n] accumulating
    PSUM[128n, 64] over a tile's 12 chunks.  S is built once per
    (layer, supertile) with a single DVE is_equal.
  * h kept transposed [64, NPC] in bf16 in SBUF across layers (no DRAM
    roundtrip); per tile: g-term matmul, fused (SU*invdeg)+g, ELU chain,
    transpose to update hT, and the next-layer table row matmul.
"""

import os
import numpy as np
import ml_dtypes

import concourse.bass as bass
import concourse.bacc as bacc
import concourse.mybir as mybir
import concourse.tile as tile
from concourse.bass_utils import run_bass_kernel_spmd
from concourse.masks import make_identity

F32 = mybir.dt.float32
BF16 = mybir.dt.bfloat16
I32 = mybir.dt.int32
I16 = mybir.dt.int16
BF_NP = ml_dtypes.bfloat16

P = 128   # partitions / tile node count / chunk edge count
D = 64    # feature dim
DP = 128  # padded feature width of the gather table (256B rows)
GMAX = 8  # max chunks (of 128 rows) per dma_gather instruction (1024 rows)


class Cfg:
    def __init__(self, n_nodes, n_cores, npc_raw, npc, spt, nb, ctb,
                 n_layers=3):
        self.n_nodes = n_nodes
        self.n_cores = n_cores
        self.npc_raw = npc_raw
        self.npc = npc                  # padded nodes per core
        self.tpc = npc // P             # tiles per core
        self.spt = spt                  # tiles per supertile
        self.nst = self.tpc // spt
        self.nb = nb                    # index buckets (core-aligned)
        self.ctb = ctb                  # chunks per (tile, bucket)
        self.cpt = nb * ctb             # chunks per tile
        self.j = spt * self.cpt         # chunks per supertile
        self.ntot = n_cores * npc
        self.bs = self.ntot // nb       # bucket size (rows); must be < 32768
        assert self.bs <= 32768
        assert (n_cores * npc) % nb == 0 and npc * (n_cores // nb) == self.bs
        self.n_layers = n_layers


def _pack_core(sizes, tpc, cap):
    """Assign nodes (rows of `sizes` [n,nb]) to tpc tiles of 128 slots s.t.
    per-tile per-bucket sums <= cap.  Greedy best-fit with a swap-repair
    pass for nodes the greedy can't place.  Returns tile index per node."""
    n, nbk = sizes.shape
    order = np.argsort(-sizes.sum(1), kind="stable")
    rem = np.full((tpc, nbk), cap, np.int64)
    slots = np.full(tpc, P)
    assign = np.full(n, -1, np.int32)
    pending = []
    for i in order:
        s = sizes[i]
        cand = (slots > 0) & np.all(rem >= s, axis=1)
        if not cand.any():
            pending.append(i)
            continue
        scores = (rem - s).min(1).astype(np.float64) + 0.001 * slots
        scores[~cand] = -1e18
        t = int(np.argmax(scores))
        rem[t] -= s
        slots[t] -= 1
        assign[i] = t
    for i in pending:
        s = sizes[i]
        placed = False
        for t in np.argsort(-(rem - s).min(1)):
            in_t = np.where(assign == t)[0]
            ok_j = np.all(rem[t][None, :] + sizes[in_t] - s[None, :] >= 0,
                          axis=1)
            for j in in_t[ok_j]:
                c2 = (slots > 0) & np.all(rem >= sizes[j], axis=1)
                c2[t] = False
                if not c2.any():
                    continue
                sc2 = (rem - sizes[j]).min(1).astype(np.float64)
                sc2[~c2] = -1e18
                t2 = int(np.argmax(sc2))
                rem[t] += sizes[j] - s
                assign[i] = t
                rem[t2] -= sizes[j]
                slots[t2] -= 1
                assign[j] = t2
                placed = True
                break
            if placed:
                break
        if not placed:
            raise RuntimeError("node packing failed; increase ctb")
    assert (assign >= 0).all()
    return assign


def prep_host(x, edge_index, Wl1, cfg: Cfg):
    N = cfg.n_nodes
    NC, NPC_RAW, NPC, TPC, SPT, NB, CTB = (
        cfg.n_cores, cfg.npc_raw, cfg.npc, cfg.tpc, cfg.spt, cfg.nb, cfg.ctb)
    NST, CPT, J = cfg.nst, cfg.cpt, cfg.j
    RCH = SPT * CTB
    BS = cfg.bs
    src = np.asarray(edge_index[0], dtype=np.int64)
    dst = np.asarray(edge_index[1], dtype=np.int64)
    E = src.shape[0]

    deg = np.bincount(src, minlength=N)
    if deg.min() == 0:
        raise NotImplementedError(
            "deg-0 nodes present; the simplified Wg+Ws fusion is invalid")
    inv_deg = (1.0 / deg).astype(np.float32)

    # Buckets are supertile-group slices (NB groups of NST/NB supertiles on
    # every core).  A node's group is fixed a priori from its raw local id,
    # so the gather table can be laid out [group][core][rows-in-group] and
    # the per-layer AllGather splits into NB pipelined slice collectives.
    NGR = NPC_RAW // NB                   # raw nodes per group
    GP = NPC // NB                        # padded positions per group
    TPG = TPC // NB                       # tiles per group
    c_src = np.minimum(src // NPC_RAW, NC - 1)
    c_dst = np.minimum(dst // NPC_RAW, NC - 1)
    loc_raw = np.arange(N) - np.minimum(np.arange(N) // NPC_RAW, NC - 1) \
        * NPC_RAW
    grp = np.minimum(loc_raw // NGR, NB - 1)
    node_core = np.minimum(np.arange(N) // NPC_RAW, NC - 1)

    # The per-(core,group) packing pools are tight (cap 25*384=9600 per
    # bucket vs ~9375 +- 84 demand), so rebalance group membership until
    # every pool-bucket demand has headroom.  Moving a node also relabels
    # its in-edges' buckets, so iterate with fresh counts each round.
    LIMIT = 9520
    for _ in range(30):
        bucket = grp[dst]
        nbcnt = np.zeros((N, NB), np.int32)
        np.add.at(nbcnt, (src, bucket), 1)
        dem = np.zeros((NC, NB, NB), np.int64)      # [core, group, bucket]
        np.add.at(dem, (node_core, grp), nbcnt)
        gsize = np.zeros((NC, NB), np.int64)
        np.add.at(gsize, (node_core, grp), 1)
        over = np.argwhere(dem.max(2) > LIMIT)
        if len(over) == 0:
            break
        for c, g in over:
            b = int(np.argmax(dem[c, g]))
            excess = int(dem[c, g, b] - (LIMIT - 50))
            pool = np.where((node_core == c) & (grp == g))[0]
            cand = pool[np.argsort(-nbcnt[pool, b])]
            moved = 0
            for v in cand[:400]:
                if moved >= excess:
                    break
                # best target: min resulting worst-bucket demand
                res = (dem[c] + nbcnt[v][None, :]).max(1).astype(np.float64)
                res[g] = 1e18
                res[gsize[c] >= NPC // NB] = 1e18
                g2 = int(np.argmin(res))
                if res[g2] > LIMIT - 10:
                    continue
                grp[v] = g2
                dem[c, g] -= nbcnt[v]
                dem[c, g2] += nbcnt[v]
                gsize[c, g] -= 1
                gsize[c, g2] += 1
                moved += nbcnt[v, b]
    else:
        bucket = grp[dst]
        nbcnt = np.zeros((N, NB), np.int32)
        np.add.at(nbcnt, (src, bucket), 1)
        dem = np.zeros((NC, NB, NB), np.int64)
        np.add.at(dem, (node_core, grp), nbcnt)
        if dem.max() > TPC // NB * CTB * P - 10:
            raise RuntimeError(f"group rebalancing plateaued at {dem.max()}")
    bucket = grp[dst]
    nbcnt = np.zeros((N, NB), np.int32)
    np.add.at(nbcnt, (src, bucket), 1)
    perm_pos = np.zeros(N, np.int64)      # orig id -> position within core
    for c in range(NC):
        lo, hi = c * NPC_RAW, min((c + 1) * NPC_RAW, N)
        ids = np.arange(lo, hi)
        for g in range(NB):
            sel = ids[grp[ids] == g]
            n_local = sel.shape[0]
            assign = _pack_core(nbcnt[sel], TPG, CTB * P)
            order_t = np.argsort(assign, kind="stable")
            within = np.arange(n_local) - np.searchsorted(
                assign[order_t], assign[order_t])
            pos = np.empty(n_local, np.int64)
            pos[order_t] = (g * TPG + assign[order_t]) * P + within
            perm_pos[sel] = pos
    gpos = np.minimum(np.arange(N) // NPC_RAW, NC - 1) * NPC + perm_pos

    # table row id in [group][core][pos-in-group] layout
    tbl_row = grp * BS + \
        np.minimum(np.arange(N) // NPC_RAW, NC - 1) * GP + (perm_pos % GP)
    pdst = tbl_row[dst]                   # row in the gather-table layout
    lsrc_tile = (perm_pos[src] % P).astype(np.int16)
    tile_of_src = perm_pos[src] // P      # tile within core
    st_of_src = tile_of_src // SPT
    t_in_st = tile_of_src % SPT

    # slot assignment: group by (core, st, bucket, tile-in-st)
    key = ((c_src * NST + st_of_src) * NB + bucket) * SPT + t_in_st
    n_groups = NC * NST * NB * SPT
    counts = np.bincount(key, minlength=n_groups)
    assert counts.max() <= CTB * P, (counts.max(), CTB * P)
    order = np.argsort(key, kind="stable")
    starts = np.zeros(n_groups + 1, np.int64)
    np.cumsum(counts, out=starts[1:])
    q = np.arange(E) - starts[key[order]]     # position within group
    ks = key[order]
    g_c = ks // (NST * NB * SPT)
    g_st = (ks // (NB * SPT)) % NST
    g_b = (ks // SPT) % NB
    g_t = ks % SPT
    chunk = g_b * RCH + g_t * CTB + q // P   # chunk within supertile
    p = q % P

    lsrc_arr = np.full((NC, NST, P, J), 300, np.int16)
    lsrc_arr[g_c, g_st, p, chunk] = lsrc_tile[order]

    # bucket-relative int16 indices; pads point at bucket row 0 (their
    # contribution is killed by S == 0)
    rel = np.zeros((NC, NST, P, J), np.int16)
    rel[g_c, g_st, p, chunk] = (pdst[order] - g_b * BS).astype(np.int16)

    # dma_gather wrapped layout: idx i (-> partition i%128, chunk i//128 of
    # the output) is read from idxs[i%16, i//16]; replicate over 8 groups.
    W16 = RCH * P // 16                   # 96 idx columns per bucket
    blocks = rel.reshape(NC, NST, P, NB, RCH)
    flat = blocks.transpose(0, 1, 3, 4, 2).reshape(NC, NST, NB, RCH * P)
    w = flat.reshape(NC, NST, NB, W16, 16).transpose(0, 1, 2, 4, 3)
    wfull = np.broadcast_to(w[:, :, :, None, :, :],
                            (NC, NST, NB, 8, 16, W16))
    gidx16 = np.ascontiguousarray(
        wfull.reshape(NC, NST, NB, P, W16).transpose(0, 1, 3, 2, 4)
        .reshape(NC, NST, P, NB * W16))

    # padded per-core node arrays (in permuted order)
    x = np.asarray(x, dtype=np.float32)
    x_pad = np.zeros((cfg.ntot, D), np.float32)
    invdeg_pad = np.zeros(cfg.ntot, np.float32)
    x_pad[gpos] = x
    invdeg_pad[gpos] = inv_deg

    # layer-0 gather table: (x @ Wl1) in bf16, 256B rows, laid out in the
    # [group][core][pos-in-group] order that matches the slice collectives
    t0 = (x @ np.asarray(Wl1, np.float32)).astype(BF_NP)
    table0 = np.zeros((cfg.ntot, DP), BF_NP)
    table0[tbl_row, :D] = t0

    per_core = []
    for c in range(NC):
        xs = x_pad[c * NPC:(c + 1) * NPC]
        per_core.append(dict(
            x_ownT=np.ascontiguousarray(xs.T.astype(BF_NP)),      # [64, NPC]
            table0=table0,                                        # [NTOT, DP]
            gidx16=np.ascontiguousarray(gidx16[c]),               # [NST,128,NB*96]
            lsrc=np.ascontiguousarray(lsrc_arr[c]),               # [NST,128,J]
            invdegT=np.ascontiguousarray(
                invdeg_pad[c * NPC:(c + 1) * NPC].reshape(TPC, P).T),
        ))
    return per_core, perm_pos


def build_program(nc, cfg: Cfg, tc=None):
    NPC, NTOT, NST, SPT, NB, CTB, J, TPC = (
        cfg.npc, cfg.ntot, cfg.nst, cfg.spt, cfg.nb, cfg.ctb, cfg.j, cfg.tpc)
    NL = cfg.n_layers
    RCH = SPT * CTB
    W16 = RCH * P // 16

    x_ownT = nc.dram_tensor("x_ownT", [D, NPC], BF16, kind="ExternalInput")
    table0 = nc.dram_tensor("table0", [NTOT, DP], BF16, kind="ExternalInput")
    gidx16 = nc.dram_tensor("gidx16", [NST, P, NB * W16], I16,
                            kind="ExternalInput")
    lsrc = nc.dram_tensor("lsrc", [NST, P, J], I16, kind="ExternalInput")
    invdegT = nc.dram_tensor("invdegT", [P, TPC], F32, kind="ExternalInput")
    w_gs = [nc.dram_tensor(f"w_gs{L}", [D, D], BF16, kind="ExternalInput")
            for L in range(NL)]
    w_ln = [nc.dram_tensor(f"w_ln{L}", [D, D], BF16, kind="ExternalInput")
            for L in range(NL - 1)]   # Wl of layer L+1
    out_own = nc.dram_tensor("out_own", [NPC, D], F32, kind="ExternalOutput")

    t2_own = [nc.dram_tensor(f"t2_own{L}", [NPC, DP], BF16, kind="Internal")
              for L in range(NL - 1)]
    s_cache = nc.dram_tensor("s_cache", [NST, P, SPT * NB * CTB * P], BF16,
                             kind="Internal")
    h_full = [nc.dram_tensor(f"h_full{L}", [NTOT, DP], BF16, kind="Internal",
                             addr_space="Shared" if cfg.n_cores > 4 else "Local")
              for L in range(NL - 1)]

    own_ctx = tc is None
    if own_ctx:
        tc = tile.TileContext(nc)
        tc.__enter__()
    try:
        _emit(nc, tc, cfg, locals())
    finally:
        if own_ctx:
            tc.__exit__(None, None, None)
    return nc


def _emit(nc, tc, cfg: Cfg, T):
    NPC, NTOT, NST, SPT, NB, CTB, J, TPC, NL = (
        cfg.npc, cfg.ntot, cfg.nst, cfg.spt, cfg.nb, cfg.ctb, cfg.j, cfg.tpc,
        cfg.n_layers)
    x_ownT, table0, gidx16, lsrc, invdegT = (
        T["x_ownT"], T["table0"], T["gidx16"], T["lsrc"], T["invdegT"])
    w_gs, w_ln, out_own = T["w_gs"], T["w_ln"], T["out_own"]
    t2_own, h_full, s_cache = T["t2_own"], T["h_full"], T["s_cache"]
    RCH = SPT * CTB               # chunks per bucket region
    BS = cfg.bs
    W16 = RCH * P // 16

    with (
        tc.tile_pool(name="const", bufs=1) as constp,
        tc.tile_pool(name="io", bufs=3) as iop,
        tc.tile_pool(name="big", bufs=2) as bigp,
        tc.tile_pool(name="small", bufs=4) as smallp,
        tc.tile_pool(name="psA", bufs=2, space="PSUM") as psA,
        tc.tile_pool(name="psB", bufs=2, space="PSUM") as psB,
        tc.tile_pool(name="psC", bufs=2, space="PSUM") as psC,
        tc.tile_pool(name="psD", bufs=2, space="PSUM") as psD,
    ):
        ident = constp.tile([P, P], BF16, name="ident")
        make_identity(nc, ident[:])
        iota16 = constp.tile([P, P], I16, name="iota16")
        nc.gpsimd.iota(iota16[:], pattern=[[1, P]], base=0, channel_multiplier=0)
        invdeg_sb = constp.tile([P, TPC], F32, name="invdeg_sb")
        nc.sync.dma_start(invdeg_sb[:], invdegT[:])
        wgs_sb, wln_sb = [], []
        for L in range(NL):
            wg_t = constp.tile([D, D], BF16, name=f"wgs_sb{L}")
            nc.sync.dma_start(wg_t[:], w_gs[L][:])
            wgs_sb.append(wg_t)
        for L in range(NL - 1):
            wl_t = constp.tile([D, D], BF16, name=f"wln_sb{L}")
            nc.sync.dma_start(wl_t[:], w_ln[L][:])
            wln_sb.append(wl_t)
        hT = [constp.tile([D, NPC], BF16, name=f"hT{i}") for i in range(2)]
        nc.sync.dma_start(hT[0][:], x_ownT[:])

        for L in range(NL):
            table = table0 if L == 0 else h_full[L - 1]
            hT_in, hT_out = hT[L % 2], hT[(L + 1) % 2]
            last = L == NL - 1
            for s in range(NST):
                idx_t = iop.tile([P, NB * W16], I16, tag="idx",
                                 name=f"idx_{L}_{s}")
                nc.sync.dma_start(idx_t[:], gidx16[s, :, :])

                X = bigp.tile([P, J * DP], BF16, tag="X", name=f"X_{L}_{s}")
                X3 = X[:].rearrange("p (c e) -> p c e", e=DP)
                # HW limit: <= 1024 gathered rows (8 chunks) per dma_gather.
                # Each queue runs on its own Q7 core pair (cpu_id/2 ==
                # queue_num in the ucode), so spreading over 4 queues
                # parallelizes descriptor generation 4x.
                gq = 0
                for b in range(NB):
                    for c0 in range(0, RCH, GMAX):
                        c1 = min(c0 + GMAX, RCH)
                        nidx = (c1 - c0) * P
                        nc.gpsimd.dma_gather(
                            out_ap=X3[:, b * RCH + c0:b * RCH + c1, :],
                            in_ap=table[b * BS:(b + 1) * BS, :],
                            idxs_ap=idx_t[:, b * W16 + c0 * (P // 16):
                                          b * W16 + c1 * (P // 16)],
                            num_idxs=nidx,
                            num_idxs_reg=nidx,
                            elem_size=DP,
                            queue_num=gq % 4,
                        )
                        gq += 1

                S = bigp.tile([P, J * P], BF16, tag="S", name=f"S_{L}_{s}")
                if L == 0:
                    # S depends only on the graph: build once, cache in DRAM
                    lsrc_t = iop.tile([P, J], I16, tag="lsrc",
                                      name=f"lsr_{L}_{s}")
                    nc.sync.dma_start(lsrc_t[:], lsrc[s, :, :])
                    nc.vector.tensor_tensor(
                        out=S[:].rearrange("p (j i) -> p j i", i=P),
                        in0=lsrc_t[:, :, None].to_broadcast([P, J, P]),
                        in1=iota16[:, None, :].to_broadcast([P, J, P]),
                        op=mybir.AluOpType.is_equal,
                    )
                    nc.sync.dma_start(s_cache[s, :, :], S[:])
                else:
                    nc.sync.dma_start(S[:], s_cache[s, :, :])

                pre_st = bigp.tile([P, SPT * D], F32, tag="pre",
                                   name=f"pre_{L}_{s}")
                if last:
                    hnew = bigp.tile([P, SPT * D], F32, tag="hnew",
                                     name=f"hn_{L}_{s}")
                else:
                    hnb = bigp.tile([P, SPT * D], BF16, tag="hnb",
                                    name=f"hb_{L}_{s}")
                    t2b = bigp.tile([P, SPT * DP], BF16, tag="t2b",
                                    name=f"t2_{L}_{s}")

                for t in range(SPT):
                    g_t = s * SPT + t
                    chunks = [b * RCH + t * CTB + k
                              for b in range(NB) for k in range(CTB)]
                    SU = psA.tile([P, D], F32, tag="SU", name=f"SU_{L}_{s}_{t}")
                    for ci, ch in enumerate(chunks):
                        nc.tensor.matmul(
                            SU[:],
                            lhsT=S[:, ch * P:(ch + 1) * P],
                            rhs=X3[:, ch, 0:D],
                            start=(ci == 0), stop=(ci == len(chunks) - 1),
                        )
                    gP = psC.tile([P, D], F32, tag="gP", name=f"gP_{L}_{s}_{t}")
                    nc.tensor.matmul(
                        gP[:], lhsT=hT_in[:, g_t * P:(g_t + 1) * P],
                        rhs=wgs_sb[L][:], start=True, stop=True)
                    # pre = SU * invdeg + g  (two ops: only one PSUM input
                    # allowed per DVE instruction)
                    e_sb = smallp.tile([P, D], F32, tag="e",
                                       name=f"e_{L}_{s}_{t}")
                    nc.vector.tensor_scalar_mul(
                        e_sb[:], SU[:], invdeg_sb[:, g_t:g_t + 1])
                    nc.vector.tensor_add(
                        pre_st[:, t * D:(t + 1) * D], e_sb[:], gP[:])

                # batched ELU over the whole supertile:
                # out = (max(x,0)-1) + exp(min(x,0))
                lo = bigp.tile([P, SPT * D], F32, tag="lo", name=f"lo_{L}_{s}")
                nc.vector.tensor_scalar_min(lo[:], pre_st[:], 0.0)
                ex = bigp.tile([P, SPT * D], F32, tag="ex", name=f"ex_{L}_{s}")
                nc.scalar.activation(ex[:], lo[:],
                                     mybir.ActivationFunctionType.Exp)
                hi1 = bigp.tile([P, SPT * D], F32, tag="hi1",
                                name=f"hi_{L}_{s}")
                nc.vector.tensor_scalar(
                    hi1[:], pre_st[:], 0.0, 1.0,
                    op0=mybir.AluOpType.max, op1=mybir.AluOpType.subtract)
                nc.vector.tensor_add(hnew[:] if last else hnb[:],
                                     ex[:], hi1[:])

                if not last:
                    for t in range(SPT):
                        g_t = s * SPT + t
                        hTP = psD.tile([D, P], BF16, tag="hTP",
                                       name=f"hTP_{L}_{s}_{t}")
                        nc.tensor.transpose(
                            hTP[:], hnb[:, t * D:(t + 1) * D], ident[:])
                        nc.vector.tensor_copy(
                            hT_out[:, g_t * P:(g_t + 1) * P], hTP[:])
                        t2P = psB.tile([P, D], F32, tag="t2P",
                                       name=f"t2P_{L}_{s}_{t}")
                        nc.tensor.matmul(
                            t2P[:], lhsT=hT_out[:, g_t * P:(g_t + 1) * P],
                            rhs=wln_sb[L][:], start=True, stop=True)
                        nc.vector.tensor_copy(
                            t2b[:, t * DP:t * DP + D], t2P[:])

                if last:
                    dst_rows = out_own.rearrange(
                        "(s t p) d -> s p t d", s=NST, t=SPT, p=P)
                    nc.sync.dma_start(
                        dst_rows[s],
                        hnew[:].rearrange("p (t d) -> p t d", d=D))
                else:
                    t2_rows = t2_own[L].rearrange(
                        "(s t p) d -> s p t d", s=NST, t=SPT, p=P)
                    nc.sync.dma_start(
                        t2_rows[s],
                        t2b[:].rearrange("p (t d) -> p t d", d=DP))
                    # supertile groups map 1:1 to table slices: fire the
                    # slice collective as soon as its group is done so the
                    # transfer overlaps the remaining groups' compute
                    if (s + 1) % (NST // NB) == 0:
                        g = s // (NST // NB)
                        GP = NPC // NB
                        nc.gpsimd.collective_compute(
                            "AllGather",
                            mybir.AluOpType.bypass,
                            replica_groups=[list(range(cfg.n_cores))],
                            ins=[t2_own[L][g * GP:(g + 1) * GP, :]],
                            outs=[h_full[L][g * BS:(g + 1) * BS, :]],
                        )


def _make_cfg_full():
    return Cfg(n_nodes=100000, n_cores=8, npc_raw=12500, npc=12800,
               spt=5, nb=4, ctb=3)


def kernel(**inputs):
    cfg = _make_cfg_full()
    x = np.asarray(inputs["x"], np.float32)
    ei = np.asarray(inputs["edge_index"])
    Wgs, Wl = [], []
    for L, (a, b, c, bias) in enumerate(
            [("Wg1", "Wl1", "Ws1", "b1"), ("Wg2", "Wl2", "Ws2", "b2"),
             ("Wgo", "Wlo", "Wso", "bo")]):
        bv = np.asarray(inputs[bias], np.float32)
        assert np.all(bv == 0.0), "nonzero bias not supported by this build"
        Wgs.append((np.asarray(inputs[a], np.float32) +
                    np.asarray(inputs[c], np.float32)).astype(BF_NP))
        Wl.append(np.asarray(inputs[b], np.float32))

    per_core, perm_pos = prep_host(x, ei, Wl[0], cfg)

    nc = bacc.Bacc("TRN2", target_bir_lowering=False, debug=False,
                   enable_asserts=False, num_devices=cfg.n_cores,
                   num_swdge_queues=4)
    build_program(nc, cfg)
    nc.compile()

    in_maps = []
    for c in range(cfg.n_cores):
        m = dict(per_core[c])
        for L in range(3):
            m[f"w_gs{L}"] = Wgs[L]
        for L in range(2):
            m[f"w_ln{L}"] = Wl[L + 1].astype(BF_NP)
        in_maps.append(m)

    res = run_bass_kernel_spmd(
        nc, in_maps, core_ids=list(range(cfg.n_cores)),
        trace=bool(int(os.environ.get("GNN_TRACE", "0"))),
    )
    full = np.zeros((cfg.n_nodes, D), np.float32)
    for c in range(cfg.n_cores):
        lo = c * cfg.npc_raw
        hi = min((c + 1) * cfg.npc_raw, cfg.n_nodes)
        full[lo:hi] = res.results[c]["out_own"][perm_pos[lo:hi]]
    kernel.last_results = res
    return full.astype(np.float32)
